# revision 17
# baseline (speedup 1.0000x reference)
"""ALSHConvNet on 8 TRN2 NeuronCores — pure data parallel (batch/8 per core).

Per core (512 samples):
- Convs as fp16 banded matmuls on TensorE, fp32 PSUM accumulation, M-order
  (parity, xpair, ch) with channel count padded to a power-of-two block so
  maxpool-x partners sit exactly 64 partitions apart (legal DVE offset).
- Weight-side hash bits AND the full layer-1 ALSH mask are computed on host;
  layer-2/3 query hashes run on DVE/ACT fully overlapped with conv matmuls.
- Mask applied once per layer on the pooled tensor (mask commutes with
  maxpool since it is a constant 0/1 per (sample, channel)).
- Maxpool: y-pairs = DVE max of the two halves of a 2-bank PSUM activation;
  x-pairs = SBUF->SBUF DMA of the upper partition half + DVE max.
- conv1 rhs = single 36-column im2col group; the ky=3,4 pass reuses the same
  SBUF data at a +3 column offset. y-edges of conv2/conv3 skip pad-ky
  matmuls; x-edges use K-trimmed weight tiles (no memsets).
- Host does layout + hashing of host-known quantities only: sharding,
  im2col, banded weights, masks, and the final [10,B] -> [B,10] transpose.
"""

import sys

for p in ("/opt/trn_rl_repo",):
    if p not in sys.path:
        sys.path.insert(0, p)

import numpy as np

import concourse.bass as bass  # noqa
import concourse.bacc as bacc
import concourse.mybir as mybir
import concourse.tile as tile
from concourse.bass_utils import run_bass_kernel_spmd

F32 = mybir.dt.float32
F16 = mybir.dt.float16
AF = mybir.ActivationFunctionType
ALU = mybir.AluOpType
AX = mybir.AxisListType

NCORES = 8
B = 512
R = 0.2
EPS = 1e-12
M_ALSH = 5

_CACHED = {}


# ---------------------------------------------------------------- host hashing
def _kernel_hash_bits(W, a, c):
    """Weight-side ALSH hash bits, fp32, mirroring reference.alsh_mask."""
    W = W.astype(np.float32)
    a = a.astype(np.float32)
    Cout = W.shape[0]
    Kf = W.reshape(Cout, -1)
    norms = np.linalg.norm(Kf, axis=1)
    Kn = Kf / (np.float32(norms.max()) + np.float32(EPS))
    n = np.linalg.norm(Kn, axis=1, keepdims=True).astype(np.float32)
    powers = np.concatenate(
        [n ** np.float32(2 ** (i + 1)) for i in range(M_ALSH)], axis=1
    ).astype(np.float32)
    P = np.concatenate([Kn, powers], axis=1)
    kh = np.mod(np.floor((P @ a + np.float32(c[0])) / np.float32(R)), 2.0)
    return kh.astype(np.float32)  # [Cout] in {0,1}


def _query_hash_bits_l1(x, a1, c1):
    """Per-sample layer-1 query hash bits, fp32, mirroring reference."""
    x = x.astype(np.float32)
    cm = x.mean(axis=(2, 3))                       # [B, 3]
    q = np.repeat(cm, 25, axis=1)                  # [B, 75]
    qn = q / (np.linalg.norm(q, axis=1, keepdims=True) + np.float32(EPS))
    Qv = np.concatenate(
        [qn, np.full((q.shape[0], M_ALSH), 0.5, np.float32)], axis=1
    )
    qh = np.mod(np.floor((Qv @ a1.astype(np.float32) + np.float32(c1[0])) / np.float32(R)), 2.0)
    return qh.astype(np.float32)  # [B] in {0,1}


# ---------------------------------------------------------------- host layout
def _band_lhsT1(W1):
    """conv1 lhsT: pass1 [108,128] (ky 0-2), pass2 [72,128] (ky 3-4).
    M-order: m = par*64 + oxp*16 + co, oxl = 2*oxp + par, wx = oxl + kx."""
    l0 = np.zeros((108, 128), np.float32)
    l1 = np.zeros((72, 128), np.float32)
    for par in range(2):
        for oxp in range(4):
            for co in range(16):
                m = par * 64 + oxp * 16 + co
                oxl = 2 * oxp + par
                for ky in range(5):
                    for ci in range(3):
                        for kx in range(5):
                            wx = oxl + kx
                            if ky < 3:
                                l0[ky * 36 + ci * 12 + wx, m] = W1[co, ci, ky, kx]
                            else:
                                l1[(ky - 3) * 36 + ci * 12 + wx, m] = W1[co, ci, ky, kx]
    return l0.astype(np.float16), l1.astype(np.float16)


def _band_lhsT2(W2):
    """conv2 lhsT per ky: [128 = wx*16+ci, 128 = par*64 + oxp*32 + co(pad32)]."""
    l = np.zeros((5, 128, 128), np.float32)
    for ky in range(5):
        for par in range(2):
            for oxp in range(2):
                for co in range(20):
                    m = par * 64 + oxp * 32 + co
                    oxl = 2 * oxp + par
                    for ci in range(16):
                        for kx in range(5):
                            l[ky, (oxl + kx) * 16 + ci, m] = W2[co, ci, ky, kx]
    return l.astype(np.float16)


def _band_lhsT3(W3):
    """conv3 lhsT per ky: [120 = wx*20+ci, 128 = par*64 + co(pad64)]."""
    l = np.zeros((5, 120, 128), np.float32)
    for ky in range(5):
        for par in range(2):
            for co in range(20):
                m = par * 64 + co
                for ci in range(20):
                    for kx in range(5):
                        l[ky, (par + kx) * 20 + ci, m] = W3[co, ci, ky, kx]
    return l.astype(np.float16)


def _fc_lhsT(Wo):
    """[64 = co(pad64), 160 = (d*4+oyp)*10 + o]; h flat idx = co*16 + oyp*4 + d."""
    l = np.zeros((64, 160), np.float32)
    for d in range(4):
        for oyp in range(4):
            for co in range(20):
                l[co, (d * 4 + oyp) * 10 : (d * 4 + oyp) * 10 + 10] = Wo[
                    :, co * 16 + oyp * 4 + d
                ]
    return l.astype(np.float16)


def _im2col1(xs):
    """g [4, 108, 36, B]: g[c][dy*36+ci*12+wx, y, b] = xpad[b, ci, y+dy, 8c+wx]."""
    xp = np.zeros((B, 3, 38, 36), np.float16)
    xp[:, :, 2:34, 2:34] = xs.astype(np.float16)
    g = np.empty((4, 108, 36, B), np.float16)
    for c in range(4):
        for dy in range(3):
            blk = xp[:, :, dy : dy + 36, 8 * c : 8 * c + 12]  # [B,3,36,12]
            g[c, dy * 36 : (dy + 1) * 36] = (
                blk.transpose(1, 3, 2, 0).reshape(36, 36, B)
            )
    return g


def _host_prep(inputs):
    x = inputs["x"].astype(np.float32)
    W1 = inputs["W1"].astype(np.float32)
    W2 = inputs["W2"].astype(np.float32)
    W3 = inputs["W3"].astype(np.float32)
    b1 = inputs["b1"].astype(np.float32)
    b2 = inputs["b2"].astype(np.float32)
    b3 = inputs["b3"].astype(np.float32)
    a1 = inputs["a1"].astype(np.float32)
    a2 = inputs["a2"].astype(np.float32)
    a3 = inputs["a3"].astype(np.float32)

    kh1 = _kernel_hash_bits(W1, a1, inputs["c1"])  # [16]
    kh2 = _kernel_hash_bits(W2, a2, inputs["c2"])  # [20]
    kh3 = _kernel_hash_bits(W3, a3, inputs["c3"])  # [20]
    qh1 = _query_hash_bits_l1(x, a1, inputs["c1"])  # [4096]
    m1 = (kh1[None, :] == qh1[:, None]).astype(np.float32)  # [4096, 16]

    l1a, l1b = _band_lhsT1(W1)
    l2 = _band_lhsT2(W2)
    l3 = _band_lhsT3(W3)

    def padco(b, n):
        o = np.zeros(n, np.float32)
        o[: b.shape[0]] = b
        return o

    b2p = padco(b2, 32)
    b3p = padco(b3, 64)
    kh2p = padco(kh2, 32)
    kh2cp = padco(1.0 - kh2, 32)
    kh3p = padco(kh3, 64)
    kh3cp = padco(1.0 - kh3, 64)

    shared = {
        "l1a": l1a,
        "l1b": l1b,
        "l2": l2,                      # [5,128,128]
        "l2e0": np.ascontiguousarray(l2[:, 32:128, :]),  # [5,96,128]
        "l3": l3,                      # [5,120,128]
        "l3e0": np.ascontiguousarray(l3[:, 40:120, :]),  # [5,80,128]
        "lo": _fc_lhsT(inputs["Wo"].astype(np.float32)),  # [64,160]
        "s2b": np.tile(np.eye(16, dtype=np.float16), (8, 1)),  # [128,16]
        "s3b": np.concatenate(
            [
                np.concatenate(
                    [np.eye(20, dtype=np.float16), np.zeros((12, 20), np.float16)],
                    axis=0,
                )
                for _ in range(4)
            ],
            axis=0,
        ),  # [128,20]
        "b1m": np.tile(b1, 8).reshape(128, 1),
        "b2m": np.tile(b2p, 4).reshape(128, 1),
        "b3m": np.tile(b3p, 2).reshape(128, 1),
        "bo": inputs["bo"].reshape(10, 1).astype(np.float32),
        "a2v": a2[:400].reshape(16, 25).sum(axis=1).reshape(16, 1),
        "a3v": a3[:500].reshape(20, 25).sum(axis=1).reshape(20, 1),
        "ones16": np.ones((16, 1), np.float32),
        "ones20": np.ones((20, 1), np.float32),
        "tc2": np.array(
            [[0.5 * a2[400:].sum() + inputs["c2"].astype(np.float32)[0]]], np.float32
        ),
        "tc3": np.array(
            [[0.5 * a3[500:].sum() + inputs["c3"].astype(np.float32)[0]]], np.float32
        ),
        "kh2t": np.tile(kh2p, 4).reshape(1, 128),
        "kh2ct": np.tile(kh2cp, 4).reshape(1, 128),
        "kh3t": np.tile(kh3p, 2).reshape(1, 128),
        "kh3ct": np.tile(kh3cp, 2).reshape(1, 128),
    }
    in_maps = []
    for i in range(NCORES):
        xs = x[i * B : (i + 1) * B]
        m = dict(shared)
        m["g"] = _im2col1(xs)
        # [128 = px8*16+ci, B] mask for H1 tiles (same pattern both tiles)
        m["mm1h"] = np.tile(m1[i * B : (i + 1) * B].T, (8, 1)).astype(np.float16)
        in_maps.append(m)
    return in_maps


# ---------------------------------------------------------------- device build
def _parity_ge1(nc, pool, t_ap, C, outtile):
    """outtile = (floor(t) mod 2) as 0/1 via fp32 magic rounding."""
    MAGIC = 12582912.0  # 1.5 * 2^23
    a = pool.tile([C, t_ap.shape[1]], F32, tag="par_a", name="par_a")
    nc.vector.tensor_scalar(a[:], t_ap, 0.5, -0.5, ALU.mult, ALU.add)
    nc.vector.tensor_scalar_add(a[:], a[:], MAGIC)
    nc.vector.tensor_scalar_add(a[:], a[:], -MAGIC)  # a = floor(t/2)
    u = pool.tile([C, t_ap.shape[1]], F32, tag="par_u", name="par_u")
    nc.vector.scalar_tensor_tensor(u[:], a[:], -2.0, t_ap, ALU.mult, ALU.add)
    nc.vector.tensor_scalar(outtile, u[:], 1.0, None, ALU.is_ge)


def build_kernel():
    nc = bacc.Bacc(None, target_bir_lowering=False, debug=False)

    def din(name, shape, dtype=F32):
        return nc.dram_tensor(name, list(shape), dtype, kind="ExternalInput").ap()

    g_in = din("g", (4, 108, 36, B), F16)
    l1a_in = din("l1a", (108, 128), F16)
    l1b_in = din("l1b", (72, 128), F16)
    l2_in = din("l2", (5, 128, 128), F16)
    l2e0_in = din("l2e0", (5, 96, 128), F16)
    l3_in = din("l3", (5, 120, 128), F16)
    l3e0_in = din("l3e0", (5, 80, 128), F16)
    lo_in = din("lo", (64, 160), F16)
    s2b_in = din("s2b", (128, 16), F16)
    s3b_in = din("s3b", (128, 20), F16)
    b1m_in = din("b1m", (128, 1))
    b2m_in = din("b2m", (128, 1))
    b3m_in = din("b3m", (128, 1))
    bo_in = din("bo", (10, 1))
    a2v_in = din("a2v", (16, 1))
    a3v_in = din("a3v", (20, 1))
    ones16_in = din("ones16", (16, 1))
    ones20_in = din("ones20", (20, 1))
    tc2_in = din("tc2", (1, 1))
    tc3_in = din("tc3", (1, 1))
    kh2t_in = din("kh2t", (1, 128))
    kh2ct_in = din("kh2ct", (1, 128))
    kh3t_in = din("kh3t", (1, 128))
    kh3ct_in = din("kh3ct", (1, 128))
    mm1h_in = din("mm1h", (128, B), F16)
    out = nc.dram_tensor("out", [10, B], F32, kind="ExternalOutput").ap()

    with tile.TileContext(nc) as tc:
        with (
            tc.tile_pool(name="const", bufs=1) as cpool,
            tc.tile_pool(name="g", bufs=3) as gpool,
            tc.tile_pool(name="h", bufs=1) as hpool,
            tc.tile_pool(name="rhs", bufs=2) as rpool,
            tc.tile_pool(name="work", bufs=2) as wpool,
            tc.tile_pool(name="q", bufs=1) as qpool,
            tc.tile_pool(name="cps", bufs=3, space="PSUM") as cps,
            tc.tile_pool(name="acc", bufs=2, space="PSUM") as acc,
        ):
            # consts go on the Scalar/Vector DMA queues so the Sync queue
            # carries only the conv1 rhs stream (+ staging later)
            def load_const(ap, dtype, tag, eng=None):
                t = cpool.tile(list(ap.shape), dtype, tag=tag, name=tag)
                (eng or nc.scalar).dma_start(t[:], ap[:])
                return t

            L1A = load_const(l1a_in, F16, "l1a")
            L1B = load_const(l1b_in, F16, "l1b")
            B1 = load_const(b1m_in, F32, "b1m")

            def load_g(c, half):
                """Half-chunk: cols y in [0,20) (half 0) or [16,36) (half 1)."""
                t = gpool.tile([108, 20 * B], F16, tag="g", name=f"g{c}_{half}")
                tv = t[:].rearrange("p (y b) -> p y b", y=20)
                yb = 16 * half
                for y0, y1 in ((0, 8), (8, 14), (14, 20)):
                    nc.sync.dma_start(tv[:, y0:y1, :], g_in[c, :, yb + y0 : yb + y1, :])
                return t

            gtiles = [load_g(0, 0), load_g(0, 1)]

            MM1H = load_const(mm1h_in, F16, "mm1h")
            S2B = load_const(s2b_in, F16, "s2b")
            S3B = load_const(s3b_in, F16, "s3b")
            B2 = load_const(b2m_in, F32, "b2m")
            B3 = load_const(b3m_in, F32, "b3m")
            BO = load_const(bo_in, F32, "bo")
            A2V = load_const(a2v_in, F32, "a2v")
            A3V = load_const(a3v_in, F32, "a3v")
            ON16 = load_const(ones16_in, F32, "ones16")
            ON20 = load_const(ones20_in, F32, "ones20")
            TC2 = load_const(tc2_in, F32, "tc2")
            TC3 = load_const(tc3_in, F32, "tc3")
            KH2T = load_const(kh2t_in, F32, "kh2t")
            KH2CT = load_const(kh2ct_in, F32, "kh2ct")
            KH3T = load_const(kh3t_in, F32, "kh3t")
            KH3CT = load_const(kh3ct_in, F32, "kh3ct")
            L2 = [load_const(l2_in[k], F16, f"l2_{k}", nc.gpsimd) for k in range(5)]
            L2E0 = [load_const(l2e0_in[k], F16, f"l2e0_{k}", nc.gpsimd) for k in range(5)]
            L3 = [load_const(l3_in[k], F16, f"l3_{k}", nc.gpsimd) for k in range(5)]
            L3E0 = [load_const(l3e0_in[k], F16, f"l3e0_{k}", nc.gpsimd) for k in range(5)]
            LO = load_const(lo_in, F16, "lo", nc.gpsimd)

            H1 = [
                hpool.tile([128, 16 * B], F16, tag=f"h1_{i}", name=f"h1_{i}")
                for i in range(2)
            ]
            H2 = [
                hpool.tile([128, 8 * B], F16, tag=f"h2_{i}", name=f"h2_{i}")
                for i in range(2)
            ]
            H3 = [
                hpool.tile([64, 4 * B], F16, tag=f"h3_{d}", name=f"h3_{d}")
                for d in range(4)
            ]

            r2 = {}

            def stage2(d):
                rhs = rpool.tile([128, 16 * B], F16, tag="rhs2", name=f"rhs2_{d}")
                if d == 0:
                    nc.sync.dma_start(rhs[0:96, :], H1[0][0:96, :])
                elif d == 1:
                    nc.sync.dma_start(rhs[0:96, :], H1[0][32:128, :])
                    nc.sync.dma_start(rhs[96:128, :], H1[1][0:32, :])
                elif d == 2:
                    nc.sync.dma_start(rhs[0:32, :], H1[0][96:128, :])
                    nc.sync.dma_start(rhs[32:128, :], H1[1][0:96, :])
                else:
                    nc.sync.dma_start(rhs[0:96, :], H1[1][32:128, :])
                r2[("c2", d)] = rhs

            def stage3(d):
                rhs = rpool.tile([128, 16 * B], F16, tag="rhs2", name=f"rhs3_{d}")
                wxs = range(2, 6) if d == 0 else (range(0, 4) if d == 3 else range(6))
                for r, wx in enumerate(wxs):
                    px3 = 2 * d - 2 + wx
                    src = H2[px3 // 4]
                    p0 = (px3 % 4) * 32
                    nc.sync.dma_start(
                        rhs[r * 20 : (r + 1) * 20, 0 : 8 * B], src[p0 : p0 + 20, :]
                    )
                r2[("c3", d)] = rhs

            cm2_ps = acc.tile([16, B], F32, tag="acc", name="cm2_ps")

            # ---------------- conv1 (+ per-tile mask & cm2 accumulation)
            for hc in range(8):
                c, half = hc // 2, hc % 2
                gt = gtiles[hc]
                if hc < 6:
                    gtiles.append(load_g((hc + 2) // 2, (hc + 2) % 2))
                for e in range(4 * half, 4 * half + 4):
                    pp = wpool.tile([128, 2 * B], F16, tag="pp", name="pp")
                    for oy2 in range(2):
                        oy = 4 * e + 2 * oy2
                        yoff = 16 * half  # tile col = y - yoff
                        ps = cps.tile([128, 2 * B], F32, tag="cps", name="cps")
                        gv = gt[:].rearrange("p (y b) -> p y b", y=20)
                        for sub in range(2):
                            nc.tensor.matmul(
                                ps[:, sub * B : (sub + 1) * B],
                                L1A[:],
                                gv[:, oy + sub - yoff, :],
                                start=True,
                                stop=False,
                            )
                            nc.tensor.matmul(
                                ps[:, sub * B : (sub + 1) * B],
                                L1B[:],
                                gv[0:72, oy + sub + 3 - yoff, :],
                                start=False,
                                stop=True,
                            )
                        a = wpool.tile([128, 2 * B], F16, tag="act", name="act")
                        nc.scalar.activation(a[:], ps[:], AF.Relu, bias=B1[:])
                        nc.vector.tensor_tensor(
                            pp[:, oy2 * B : (oy2 + 1) * B],
                            a[:, 0:B],
                            a[:, B : 2 * B],
                            ALU.max,
                        )
                    mv = wpool.tile([64, 2 * B], F16, tag="mv", name="mv")
                    nc.gpsimd.dma_start(mv[:], pp[64:128, :])
                    # chunk c covers px 4c..4c+3 -> H1[c//2] at offset (c%2)*64
                    nc.vector.tensor_tensor(
                        H1[c // 2][
                            (c % 2) * 64 : (c % 2) * 64 + 64,
                            2 * e * B : (2 * e + 2) * B,
                        ],
                        pp[0:64, :],
                        mv[:],
                        ALU.max,
                    )
                if hc % 4 == 3:
                    i = hc // 4
                    h1v = H1[i][:].rearrange("p (y b) -> p y b", y=16)
                    mb = MM1H[:].broadcast_to((128, B, 16)).rearrange("p b y -> p y b")
                    nc.vector.tensor_tensor(h1v, h1v, mb, ALU.mult)
                    for oy in range(16):
                        nc.tensor.matmul(
                            cm2_ps[:],
                            S2B[:],
                            H1[i][:, oy * B : (oy + 1) * B],
                            start=(i == 0 and oy == 0),
                            stop=(i == 1 and oy == 15),
                            skip_group_check=True,
                        )
                    stage2(i)

            # ---------------- query hash chain (stage A: uses PE right away;
            # stage B's PE ops are emitted a d-iteration later so the DVE/ACT
            # part of the chain hides under conv matmuls without stalling PE)
            def qchain_a(cmps_ap, C, AV, ONESC, TC, lname):
                cmsb = qpool.tile([C, B], F32, tag="q_cmsb", name=f"cmsb{lname}")
                nc.scalar.activation(cmsb[:], cmps_ap, AF.Identity)
                sq = qpool.tile([C, B], F32, tag="q_sq", name=f"sq{lname}")
                nc.scalar.activation(sq[:], cmps_ap, AF.Square)
                num_ps = acc.tile([1, B], F32, tag="acc", name=f"num{lname}")
                nc.tensor.matmul(num_ps[:], AV[:, 0:1], cmsb[:], start=True, stop=True)
                den_ps = acc.tile([1, B], F32, tag="acc", name=f"den{lname}")
                nc.tensor.matmul(den_ps[:], ONESC[:, 0:1], sq[:], start=True, stop=True)
                den = qpool.tile([1, B], F32, tag="q_den", name=f"den{lname}")
                nc.scalar.activation(den[:], den_ps[:], AF.Sqrt, scale=25.0)
                nc.vector.tensor_scalar_add(den[:], den[:], EPS)
                rden = qpool.tile([1, B], F32, tag="q_rden", name=f"rden{lname}")
                nc.vector.reciprocal(rden[:], den[:])
                nums = qpool.tile([1, B], F32, tag="q_nums", name=f"nums{lname}")
                nc.scalar.activation(nums[:], num_ps[:], AF.Identity)
                qv = qpool.tile([1, B], F32, tag="q_qv", name=f"qv{lname}")
                nc.vector.tensor_tensor(qv[:], nums[:], rden[:], ALU.mult)
                nc.vector.tensor_scalar(qv[:], qv[:], TC[0:1, 0:1], 1.0 / R, ALU.add, ALU.mult)
                qh = qpool.tile([1, B], F32, tag="q_qh", name=f"qh{lname}")
                _parity_ge1(nc, qpool, qv[:], 1, qh[:])
                qc = qpool.tile([1, B], F32, tag="q_qc", name=f"qc{lname}")
                nc.vector.tensor_scalar(qc[:], qh[:], -1.0, 1.0, ALU.mult, ALU.add)
                return qh, qc

            def qchain_b(qh, qc, KHT, KHCT, lname):
                map_ps = acc.tile([128, B], F32, tag="acc", name=f"map{lname}")
                nc.tensor.matmul(map_ps[:], KHT[0:1, :], qh[:], start=True, stop=False)
                nc.tensor.matmul(map_ps[:], KHCT[0:1, :], qc[:], start=False, stop=True)
                mm = hpool.tile([128, B], F16, tag=f"mm{lname}", name=f"mm{lname}")
                nc.scalar.activation(mm[:], map_ps[:], AF.Identity)
                return mm

            # ---------------- conv2
            qh2, qc2 = qchain_a(cm2_ps[:], 16, A2V, ON16, TC2, "2")
            cm3_ps = None
            mm2 = None
            for d in range(4):
                nk = 96 if d in (0, 3) else 128
                rhs = r2[("c2", d)]

                def lhs2(ky):
                    if d == 0:
                        return L2E0[ky][:]
                    if d == 3:
                        return L2[ky][0:96, :]
                    return L2[ky][:]

                rv = rhs[0:nk, :].rearrange("p (y b) -> p y b", y=16)
                for oy2 in range(8):
                    ps = cps.tile([128, 2 * B], F32, tag="cps", name="cps")
                    for sub in range(2):
                        oy = 2 * oy2 + sub
                        kys = [k for k in range(5) if 0 <= oy + k - 2 < 16]
                        for j, ky in enumerate(kys):
                            nc.tensor.matmul(
                                ps[:, sub * B : (sub + 1) * B],
                                lhs2(ky),
                                rv[:, oy + ky - 2, :],
                                start=(j == 0),
                                stop=(j == len(kys) - 1),
                            )
                    a = wpool.tile([128, 2 * B], F16, tag="act", name="act")
                    nc.scalar.activation(a[:], ps[:], AF.Relu, bias=B2[:])
                    if oy2 % 2 == 0:
                        pp = wpool.tile([128, 2 * B], F16, tag="pp", name="pp")
                    nc.vector.tensor_tensor(
                        pp[:, (oy2 % 2) * B : (oy2 % 2 + 1) * B],
                        a[:, 0:B],
                        a[:, B : 2 * B],
                        ALU.max,
                    )
                    if oy2 % 2 == 1:
                        mv = wpool.tile([64, 2 * B], F16, tag="mv", name="mv")
                        nc.gpsimd.dma_start(mv[:], pp[64:128, :])
                        j2 = oy2 // 2  # pooled-row pair index
                        nc.vector.tensor_tensor(
                            H2[d // 2][
                                (d % 2) * 64 : (d % 2) * 64 + 64,
                                2 * j2 * B : (2 * j2 + 2) * B,
                            ],
                            pp[0:64, :],
                            mv[:],
                            ALU.max,
                        )
                if d == 0:
                    stage2(2)
                    mm2 = qchain_b(qh2, qc2, KH2T, KH2CT, "2")
                if d == 1:
                    stage2(3)
                if d % 2 == 1:
                    i = d // 2
                    h2v = H2[i][:].rearrange("p (y b) -> p y b", y=8)
                    mb = mm2[:].broadcast_to((128, B, 8)).rearrange("p b y -> p y b")
                    nc.vector.tensor_tensor(h2v, h2v, mb, ALU.mult)
                    if cm3_ps is None:
                        cm3_ps = acc.tile([20, B], F32, tag="acc", name="cm3_ps")
                    for oy in range(8):
                        nc.tensor.matmul(
                            cm3_ps[:],
                            S3B[:],
                            H2[i][:, oy * B : (oy + 1) * B],
                            start=(i == 0 and oy == 0),
                            stop=(i == 1 and oy == 7),
                            skip_group_check=True,
                        )
                    stage3(i)

            # ---------------- conv3
            qh3, qc3 = qchain_a(cm3_ps[:], 20, A3V, ON20, TC3, "3")
            fc_ps = acc.tile([10, B], F32, tag="acc", name="fc_ps")
            mm3 = None
            for d in range(4):
                nk = 80 if d in (0, 3) else 120
                rhs = r2[("c3", d)]

                def lhs3(ky):
                    if d == 0:
                        return L3E0[ky][:]
                    if d == 3:
                        return L3[ky][0:80, :]
                    return L3[ky][:]

                rv = rhs[0:nk, 0 : 8 * B].rearrange("p (y b) -> p y b", y=8)
                for oy2 in range(4):
                    ps = cps.tile([128, 2 * B], F32, tag="cps", name="cps")
                    for sub in range(2):
                        oy = 2 * oy2 + sub
                        kys = [k for k in range(5) if 0 <= oy + k - 2 < 8]
                        for j, ky in enumerate(kys):
                            nc.tensor.matmul(
                                ps[:, sub * B : (sub + 1) * B],
                                lhs3(ky),
                                rv[:, oy + ky - 2, :],
                                start=(j == 0),
                                stop=(j == len(kys) - 1),
                            )
                    a = wpool.tile([128, 2 * B], F16, tag="act", name="act")
                    nc.scalar.activation(a[:], ps[:], AF.Relu, bias=B3[:])
                    if oy2 % 2 == 0:
                        pp = wpool.tile([128, 2 * B], F16, tag="pp", name="pp")
                    nc.vector.tensor_tensor(
                        pp[:, (oy2 % 2) * B : (oy2 % 2 + 1) * B],
                        a[:, 0:B],
                        a[:, B : 2 * B],
                        ALU.max,
                    )
                    if oy2 % 2 == 1:
                        mv = wpool.tile([64, 2 * B], F16, tag="mv", name="mv")
                        nc.gpsimd.dma_start(mv[:], pp[64:128, :])
                        j2 = oy2 // 2
                        nc.vector.tensor_tensor(
                            H3[d][:, 2 * j2 * B : (2 * j2 + 2) * B],
                            pp[0:64, :],
                            mv[:],
                            ALU.max,
                        )
                if d == 0:
                    stage3(2)
                    mm3 = qchain_b(qh3, qc3, KH3T, KH3CT, "3")
                if d == 1:
                    stage3(3)
                # mask + FC accumulation for this d-chunk
                h3v = H3[d][:].rearrange("p (y b) -> p y b", y=4)
                mb = mm3[0:64, :].broadcast_to((64, B, 4)).rearrange("p b y -> p y b")
                nc.vector.tensor_tensor(h3v, h3v, mb, ALU.mult)
                for oyp in range(4):
                    nc.tensor.matmul(
                        fc_ps[:],
                        LO[:, (d * 4 + oyp) * 10 : (d * 4 + oyp) * 10 + 10],
                        H3[d][:, oyp * B : (oyp + 1) * B],
                        start=(d == 0 and oyp == 0),
                        stop=(d == 3 and oyp == 3),
                        skip_group_check=True,
                    )

            ob = qpool.tile([10, B], F32, tag="outb", name="outb")
            nc.scalar.activation(ob[:], fc_ps[:], AF.Identity, bias=BO[:])
            nc.sync.dma_start(out[:], ob[:])

    nc.compile()
    return nc


# ---------------------------------------------------------------- entry point
def kernel(**inputs) -> np.ndarray:
    in_maps = _host_prep(inputs)
    if "nc" not in _CACHED:
        _CACHED["nc"] = build_kernel()
    nc = _CACHED["nc"]
    res = run_bass_kernel_spmd(nc, in_maps, core_ids=list(range(NCORES)))
    outs = [res.results[i]["out"].T for i in range(NCORES)]
    return np.ascontiguousarray(np.concatenate(outs, axis=0)).astype(np.float32)


# revision 20
# speedup vs baseline: 1.0924x; 1.0924x over previous
"""ALSHConvNet on 8 TRN2 NeuronCores — pure data parallel (batch/8 per core).

Per core (512 samples):
- Convs as fp16 banded matmuls on TensorE, fp32 PSUM accumulation, M-order
  (parity, xpair, ch) with channel count padded to a power-of-two block so
  maxpool-x partners sit exactly 64 partitions apart (legal DVE offset).
- Weight-side hash bits AND the full layer-1 ALSH mask are computed on host;
  layer-2/3 query hashes run on DVE/ACT fully overlapped with conv matmuls.
- Mask applied once per layer on the pooled tensor (mask commutes with
  maxpool since it is a constant 0/1 per (sample, channel)).
- Maxpool: y-pairs = DVE max of the two halves of a 2-bank PSUM activation;
  x-pairs = SBUF->SBUF DMA of the upper partition half + DVE max.
- conv1 rhs = single 36-column im2col group; the ky=3,4 pass reuses the same
  SBUF data at a +3 column offset. y-edges of conv2/conv3 skip pad-ky
  matmuls; x-edges use K-trimmed weight tiles (no memsets).
- Host does layout + hashing of host-known quantities only: sharding,
  im2col, banded weights, masks, and the final [10,B] -> [B,10] transpose.
"""

import sys

for p in ("/opt/trn_rl_repo",):
    if p not in sys.path:
        sys.path.insert(0, p)

import numpy as np

import concourse.bass as bass  # noqa
import concourse.bacc as bacc
import concourse.mybir as mybir
import concourse.tile as tile
from concourse.bass_utils import run_bass_kernel_spmd

F32 = mybir.dt.float32
F16 = mybir.dt.float16
AF = mybir.ActivationFunctionType
ALU = mybir.AluOpType
AX = mybir.AxisListType

NCORES = 8
B = 512
R = 0.2
EPS = 1e-12
M_ALSH = 5

_CACHED = {}


# ---------------------------------------------------------------- host hashing
def _kernel_hash_bits(W, a, c):
    """Weight-side ALSH hash bits, fp32, mirroring reference.alsh_mask."""
    W = W.astype(np.float32)
    a = a.astype(np.float32)
    Cout = W.shape[0]
    Kf = W.reshape(Cout, -1)
    norms = np.linalg.norm(Kf, axis=1)
    Kn = Kf / (np.float32(norms.max()) + np.float32(EPS))
    n = np.linalg.norm(Kn, axis=1, keepdims=True).astype(np.float32)
    powers = np.concatenate(
        [n ** np.float32(2 ** (i + 1)) for i in range(M_ALSH)], axis=1
    ).astype(np.float32)
    P = np.concatenate([Kn, powers], axis=1)
    kh = np.mod(np.floor((P @ a + np.float32(c[0])) / np.float32(R)), 2.0)
    return kh.astype(np.float32)  # [Cout] in {0,1}


def _query_hash_bits_l1(x, a1, c1):
    """Per-sample layer-1 query hash bits, fp32, mirroring reference."""
    x = x.astype(np.float32)
    cm = x.mean(axis=(2, 3))                       # [B, 3]
    q = np.repeat(cm, 25, axis=1)                  # [B, 75]
    qn = q / (np.linalg.norm(q, axis=1, keepdims=True) + np.float32(EPS))
    Qv = np.concatenate(
        [qn, np.full((q.shape[0], M_ALSH), 0.5, np.float32)], axis=1
    )
    qh = np.mod(np.floor((Qv @ a1.astype(np.float32) + np.float32(c1[0])) / np.float32(R)), 2.0)
    return qh.astype(np.float32)  # [B] in {0,1}


# ---------------------------------------------------------------- host layout
def _band_lhsT1(W1):
    """conv1 lhsT: pass1 [108,128] (ky 0-2), pass2 [72,128] (ky 3-4).
    M-order: m = par*64 + oxp*16 + co, oxl = 2*oxp + par, wx = oxl + kx."""
    l0 = np.zeros((108, 128), np.float32)
    l1 = np.zeros((72, 128), np.float32)
    for par in range(2):
        for oxp in range(4):
            for co in range(16):
                m = par * 64 + oxp * 16 + co
                oxl = 2 * oxp + par
                for ky in range(5):
                    for ci in range(3):
                        for kx in range(5):
                            wx = oxl + kx
                            if ky < 3:
                                l0[ky * 36 + ci * 12 + wx, m] = W1[co, ci, ky, kx]
                            else:
                                l1[(ky - 3) * 36 + ci * 12 + wx, m] = W1[co, ci, ky, kx]
    return l0.astype(np.float16), l1.astype(np.float16)


def _band_lhsT2(W2):
    """conv2 lhsT per ky: [128 = wx*16+ci, 128 = par*64 + oxp*32 + co(pad32)]."""
    l = np.zeros((5, 128, 128), np.float32)
    for ky in range(5):
        for par in range(2):
            for oxp in range(2):
                for co in range(20):
                    m = par * 64 + oxp * 32 + co
                    oxl = 2 * oxp + par
                    for ci in range(16):
                        for kx in range(5):
                            l[ky, (oxl + kx) * 16 + ci, m] = W2[co, ci, ky, kx]
    return l.astype(np.float16)


def _band_lhsT3(W3):
    """conv3 lhsT per ky: [120 = wx*20+ci, 128 = par*64 + co(pad64)]."""
    l = np.zeros((5, 120, 128), np.float32)
    for ky in range(5):
        for par in range(2):
            for co in range(20):
                m = par * 64 + co
                for ci in range(20):
                    for kx in range(5):
                        l[ky, (par + kx) * 20 + ci, m] = W3[co, ci, ky, kx]
    return l.astype(np.float16)


def _fc_lhsT(Wo):
    """[64 = co(pad64), 160 = (d*4+oyp)*10 + o]; h flat idx = co*16 + oyp*4 + d."""
    l = np.zeros((64, 160), np.float32)
    for d in range(4):
        for oyp in range(4):
            for co in range(20):
                l[co, (d * 4 + oyp) * 10 : (d * 4 + oyp) * 10 + 10] = Wo[
                    :, co * 16 + oyp * 4 + d
                ]
    return l.astype(np.float16)


def _im2col1(xs):
    """g [4, 108, 36, B]: g[c][dy*36+ci*12+wx, y, b] = xpad[b, ci, y+dy, 8c+wx]."""
    xp = np.zeros((B, 3, 38, 36), np.float16)
    xp[:, :, 2:34, 2:34] = xs.astype(np.float16)
    g = np.empty((4, 108, 36, B), np.float16)
    for c in range(4):
        for dy in range(3):
            blk = xp[:, :, dy : dy + 36, 8 * c : 8 * c + 12]  # [B,3,36,12]
            g[c, dy * 36 : (dy + 1) * 36] = (
                blk.transpose(1, 3, 2, 0).reshape(36, 36, B)
            )
    return g


def _host_prep(inputs):
    x = inputs["x"].astype(np.float32)
    W1 = inputs["W1"].astype(np.float32)
    W2 = inputs["W2"].astype(np.float32)
    W3 = inputs["W3"].astype(np.float32)
    b1 = inputs["b1"].astype(np.float32)
    b2 = inputs["b2"].astype(np.float32)
    b3 = inputs["b3"].astype(np.float32)
    a1 = inputs["a1"].astype(np.float32)
    a2 = inputs["a2"].astype(np.float32)
    a3 = inputs["a3"].astype(np.float32)

    kh1 = _kernel_hash_bits(W1, a1, inputs["c1"])  # [16]
    kh2 = _kernel_hash_bits(W2, a2, inputs["c2"])  # [20]
    kh3 = _kernel_hash_bits(W3, a3, inputs["c3"])  # [20]
    qh1 = _query_hash_bits_l1(x, a1, inputs["c1"])  # [4096]
    m1 = (kh1[None, :] == qh1[:, None]).astype(np.float32)  # [4096, 16]

    l1a, l1b = _band_lhsT1(W1)
    l2 = _band_lhsT2(W2)
    l3 = _band_lhsT3(W3)

    def padco(b, n):
        o = np.zeros(n, np.float32)
        o[: b.shape[0]] = b
        return o

    b2p = padco(b2, 32)
    b3p = padco(b3, 64)
    kh2p = padco(kh2, 32)
    kh2cp = padco(1.0 - kh2, 32)
    kh3p = padco(kh3, 64)
    kh3cp = padco(1.0 - kh3, 64)

    shared = {
        "l1a": l1a,
        "l1b": l1b,
        "l2": l2,                      # [5,128,128]
        "l2e0": np.ascontiguousarray(l2[:, 32:128, :]),  # [5,96,128]
        "l3": l3,                      # [5,120,128]
        "l3e0": np.ascontiguousarray(l3[:, 40:120, :]),  # [5,80,128]
        "lo": _fc_lhsT(inputs["Wo"].astype(np.float32)),  # [64,160]
        "s2b": np.tile(np.eye(16, dtype=np.float16), (8, 1)),  # [128,16]
        "s3b": np.concatenate(
            [
                np.concatenate(
                    [np.eye(20, dtype=np.float16), np.zeros((12, 20), np.float16)],
                    axis=0,
                )
                for _ in range(4)
            ],
            axis=0,
        ),  # [128,20]
        "b1m": np.tile(b1, 8).reshape(128, 1),
        "b2m": np.tile(b2p, 4).reshape(128, 1),
        "b3m": np.tile(b3p, 2).reshape(128, 1),
        "bo": inputs["bo"].reshape(10, 1).astype(np.float32),
        "a2v": a2[:400].reshape(16, 25).sum(axis=1).reshape(16, 1),
        "a3v": a3[:500].reshape(20, 25).sum(axis=1).reshape(20, 1),
        "ones16": np.ones((16, 1), np.float32),
        "ones20": np.ones((20, 1), np.float32),
        "tc2": np.array(
            [[0.5 * a2[400:].sum() + inputs["c2"].astype(np.float32)[0]]], np.float32
        ),
        "tc3": np.array(
            [[0.5 * a3[500:].sum() + inputs["c3"].astype(np.float32)[0]]], np.float32
        ),
        "kh2t": np.tile(kh2p, 4).reshape(1, 128),
        "kh2ct": np.tile(kh2cp, 4).reshape(1, 128),
        "kh3t": np.tile(kh3p, 2).reshape(1, 128),
        "kh3ct": np.tile(kh3cp, 2).reshape(1, 128),
    }
    in_maps = []
    for i in range(NCORES):
        xs = x[i * B : (i + 1) * B]
        m = dict(shared)
        m["g"] = _im2col1(xs)
        # [128 = px8*16+ci, B] mask for H1 tiles (same pattern both tiles)
        m["mm1h"] = np.tile(m1[i * B : (i + 1) * B].T, (8, 1)).astype(np.float16)
        in_maps.append(m)
    return in_maps


# ---------------------------------------------------------------- device build
def _parity_ge1(nc, pool, t_ap, C, outtile):
    """outtile = (floor(t) mod 2) as 0/1 via fp32 magic rounding."""
    MAGIC = 12582912.0  # 1.5 * 2^23
    a = pool.tile([C, t_ap.shape[1]], F32, tag="par_a", name="par_a")
    nc.vector.tensor_scalar(a[:], t_ap, 0.5, -0.5, ALU.mult, ALU.add)
    nc.vector.tensor_scalar_add(a[:], a[:], MAGIC)
    nc.vector.tensor_scalar_add(a[:], a[:], -MAGIC)  # a = floor(t/2)
    u = pool.tile([C, t_ap.shape[1]], F32, tag="par_u", name="par_u")
    nc.vector.scalar_tensor_tensor(u[:], a[:], -2.0, t_ap, ALU.mult, ALU.add)
    nc.vector.tensor_scalar(outtile, u[:], 1.0, None, ALU.is_ge)


def build_kernel():
    nc = bacc.Bacc(None, target_bir_lowering=False, debug=False)

    def din(name, shape, dtype=F32):
        return nc.dram_tensor(name, list(shape), dtype, kind="ExternalInput").ap()

    g_in = din("g", (4, 108, 36, B), F16)
    l1a_in = din("l1a", (108, 128), F16)
    l1b_in = din("l1b", (72, 128), F16)
    l2_in = din("l2", (5, 128, 128), F16)
    l2e0_in = din("l2e0", (5, 96, 128), F16)
    l3_in = din("l3", (5, 120, 128), F16)
    l3e0_in = din("l3e0", (5, 80, 128), F16)
    lo_in = din("lo", (64, 160), F16)
    s2b_in = din("s2b", (128, 16), F16)
    s3b_in = din("s3b", (128, 20), F16)
    b1m_in = din("b1m", (128, 1))
    b2m_in = din("b2m", (128, 1))
    b3m_in = din("b3m", (128, 1))
    bo_in = din("bo", (10, 1))
    a2v_in = din("a2v", (16, 1))
    a3v_in = din("a3v", (20, 1))
    ones16_in = din("ones16", (16, 1))
    ones20_in = din("ones20", (20, 1))
    tc2_in = din("tc2", (1, 1))
    tc3_in = din("tc3", (1, 1))
    kh2t_in = din("kh2t", (1, 128))
    kh2ct_in = din("kh2ct", (1, 128))
    kh3t_in = din("kh3t", (1, 128))
    kh3ct_in = din("kh3ct", (1, 128))
    mm1h_in = din("mm1h", (128, B), F16)
    out = nc.dram_tensor("out", [10, B], F32, kind="ExternalOutput").ap()

    with tile.TileContext(nc) as tc:
        with (
            tc.tile_pool(name="const", bufs=1) as cpool,
            tc.tile_pool(name="g", bufs=3) as gpool,
            tc.tile_pool(name="h", bufs=1) as hpool,
            tc.tile_pool(name="rhs", bufs=2) as rpool,
            tc.tile_pool(name="apool", bufs=3) as apool,
            tc.tile_pool(name="ppool", bufs=3) as ppool,
            tc.tile_pool(name="mvpool", bufs=2) as mvpool,
            tc.tile_pool(name="q", bufs=1) as qpool,
            tc.tile_pool(name="cps", bufs=3, space="PSUM") as cps,
            tc.tile_pool(name="acc", bufs=2, space="PSUM") as acc,
        ):
            def load_const(ap, dtype, tag, eng=None):
                t = cpool.tile(list(ap.shape), dtype, tag=tag, name=tag)
                (eng or nc.scalar).dma_start(t[:], ap[:])
                return t

            L1A = load_const(l1a_in, F16, "l1a")
            L1B = load_const(l1b_in, F16, "l1b")
            B1 = load_const(b1m_in, F32, "b1m")

            def load_g(c, half):
                """Half-chunk: cols y in [0,20) (half 0) or [16,36) (half 1)."""
                t = gpool.tile([108, 20 * B], F16, tag="g", name=f"g{c}_{half}")
                tv = t[:].rearrange("p (y b) -> p y b", y=20)
                yb = 16 * half
                for y0, y1 in ((0, 8), (8, 14), (14, 20)):
                    nc.sync.dma_start(tv[:, y0:y1, :], g_in[c, :, yb + y0 : yb + y1, :])
                return t

            gtiles = [load_g(0, 0), load_g(0, 1)]

            MM1H = load_const(mm1h_in, F16, "mm1h")
            S2B = load_const(s2b_in, F16, "s2b")
            S3B = load_const(s3b_in, F16, "s3b")
            B2 = load_const(b2m_in, F32, "b2m")
            B3 = load_const(b3m_in, F32, "b3m")
            BO = load_const(bo_in, F32, "bo")
            A2V = load_const(a2v_in, F32, "a2v")
            A3V = load_const(a3v_in, F32, "a3v")
            ON16 = load_const(ones16_in, F32, "ones16")
            ON20 = load_const(ones20_in, F32, "ones20")
            TC2 = load_const(tc2_in, F32, "tc2")
            TC3 = load_const(tc3_in, F32, "tc3")
            KH2T = load_const(kh2t_in, F32, "kh2t")
            KH2CT = load_const(kh2ct_in, F32, "kh2ct")
            KH3T = load_const(kh3t_in, F32, "kh3t")
            KH3CT = load_const(kh3ct_in, F32, "kh3ct")
            L2 = [load_const(l2_in[k], F16, f"l2_{k}", nc.gpsimd) for k in range(5)]
            L2E0 = [load_const(l2e0_in[k], F16, f"l2e0_{k}", nc.gpsimd) for k in range(5)]
            L3 = [load_const(l3_in[k], F16, f"l3_{k}", nc.gpsimd) for k in range(5)]
            L3E0 = [load_const(l3e0_in[k], F16, f"l3e0_{k}", nc.gpsimd) for k in range(5)]
            LO = load_const(lo_in, F16, "lo", nc.gpsimd)

            H1 = [
                hpool.tile([128, 16 * B], F16, tag=f"h1_{i}", name=f"h1_{i}")
                for i in range(2)
            ]
            H2 = [
                hpool.tile([128, 8 * B], F16, tag=f"h2_{i}", name=f"h2_{i}")
                for i in range(2)
            ]
            H3 = [
                hpool.tile([64, 4 * B], F16, tag=f"h3_{d}", name=f"h3_{d}")
                for d in range(4)
            ]

            r2 = {}

            def stage2(d):
                rhs = rpool.tile([128, 16 * B], F16, tag="rhs2", name=f"rhs2_{d}")
                if d == 0:
                    nc.sync.dma_start(rhs[0:96, :], H1[0][0:96, :])
                elif d == 1:
                    nc.sync.dma_start(rhs[0:96, :], H1[0][32:128, :])
                    nc.sync.dma_start(rhs[96:128, :], H1[1][0:32, :])
                elif d == 2:
                    nc.sync.dma_start(rhs[0:32, :], H1[0][96:128, :])
                    nc.sync.dma_start(rhs[32:128, :], H1[1][0:96, :])
                else:
                    nc.sync.dma_start(rhs[0:96, :], H1[1][32:128, :])
                r2[("c2", d)] = rhs

            def stage3(d):
                rhs = rpool.tile([128, 16 * B], F16, tag="rhs2", name=f"rhs3_{d}")
                wxs = range(2, 6) if d == 0 else (range(0, 4) if d == 3 else range(6))
                for r, wx in enumerate(wxs):
                    px3 = 2 * d - 2 + wx
                    src = H2[px3 // 4]
                    p0 = (px3 % 4) * 32
                    nc.sync.dma_start(
                        rhs[r * 20 : (r + 1) * 20, 0 : 8 * B], src[p0 : p0 + 20, :]
                    )
                r2[("c3", d)] = rhs

            def mask_h1_piece(i, j):
                """Multiply mask into H1[i] cols [4j,4j+4) (4 of 16 oy)."""
                h1v = H1[i][:, 4 * j * B : (4 * j + 4) * B].rearrange(
                    "p (y b) -> p y b", y=4
                )
                mb = MM1H[:].broadcast_to((128, B, 4)).rearrange("p b y -> p y b")
                nc.vector.tensor_tensor(h1v, h1v, mb, ALU.mult)

            def cm2_block(i, cm2_ps):
                for oy in range(16):
                    nc.tensor.matmul(
                        cm2_ps[:],
                        S2B[:],
                        H1[i][:, oy * B : (oy + 1) * B],
                        start=(i == 0 and oy == 0),
                        stop=(i == 1 and oy == 15),
                        skip_group_check=True,
                    )

            def mask_h2(i, mm2):
                for j in range(2):
                    h2v = H2[i][:, 4 * j * B : (4 * j + 4) * B].rearrange(
                        "p (y b) -> p y b", y=4
                    )
                    mb = mm2[:].broadcast_to((128, B, 4)).rearrange("p b y -> p y b")
                    nc.vector.tensor_tensor(h2v, h2v, mb, ALU.mult)

            def cm3_block(i, cm3_ps):
                for oy in range(8):
                    nc.tensor.matmul(
                        cm3_ps[:],
                        S3B[:],
                        H2[i][:, oy * B : (oy + 1) * B],
                        start=(i == 0 and oy == 0),
                        stop=(i == 1 and oy == 7),
                        skip_group_check=True,
                    )

            def qchain_a(cmps_ap, C, AV, ONESC, TC, lname):
                cmsb = qpool.tile([C, B], F32, tag="q_cmsb", name=f"cmsb{lname}")
                nc.scalar.activation(cmsb[:], cmps_ap, AF.Identity)
                sq = qpool.tile([C, B], F32, tag="q_sq", name=f"sq{lname}")
                nc.scalar.activation(sq[:], cmps_ap, AF.Square)
                num_ps = acc.tile([1, B], F32, tag="acc", name=f"num{lname}")
                nc.tensor.matmul(num_ps[:], AV[:, 0:1], cmsb[:], start=True, stop=True)
                den_ps = acc.tile([1, B], F32, tag="acc", name=f"den{lname}")
                nc.tensor.matmul(den_ps[:], ONESC[:, 0:1], sq[:], start=True, stop=True)
                den = qpool.tile([1, B], F32, tag="q_den", name=f"den{lname}")
                nc.scalar.activation(den[:], den_ps[:], AF.Sqrt, scale=25.0)
                nc.vector.tensor_scalar_add(den[:], den[:], EPS)
                rden = qpool.tile([1, B], F32, tag="q_rden", name=f"rden{lname}")
                nc.vector.reciprocal(rden[:], den[:])
                nums = qpool.tile([1, B], F32, tag="q_nums", name=f"nums{lname}")
                nc.scalar.activation(nums[:], num_ps[:], AF.Identity)
                qv = qpool.tile([1, B], F32, tag="q_qv", name=f"qv{lname}")
                nc.vector.tensor_tensor(qv[:], nums[:], rden[:], ALU.mult)
                nc.vector.tensor_scalar(qv[:], qv[:], TC[0:1, 0:1], 1.0 / R, ALU.add, ALU.mult)
                qh = qpool.tile([1, B], F32, tag="q_qh", name=f"qh{lname}")
                _parity_ge1(nc, qpool, qv[:], 1, qh[:])
                qc = qpool.tile([1, B], F32, tag="q_qc", name=f"qc{lname}")
                nc.vector.tensor_scalar(qc[:], qh[:], -1.0, 1.0, ALU.mult, ALU.add)
                return qh, qc

            def qchain_b(qh, qc, KHT, KHCT, lname):
                map_ps = acc.tile([128, B], F32, tag="acc", name=f"map{lname}")
                nc.tensor.matmul(map_ps[:], KHT[0:1, :], qh[:], start=True, stop=False)
                nc.tensor.matmul(map_ps[:], KHCT[0:1, :], qc[:], start=False, stop=True)
                mm = hpool.tile([128, B], F16, tag=f"mm{lname}", name=f"mm{lname}")
                nc.scalar.activation(mm[:], map_ps[:], AF.Identity)
                return mm

            pend = [None]  # delayed x-fold: (dst_ap, pp, mv)

            def flush_fold():
                if pend[0] is not None:
                    dst, fpp, fmv = pend[0]
                    nc.vector.tensor_tensor(dst, fpp[0:64, :], fmv[:], ALU.max)
                    pend[0] = None

            cm2_ps = acc.tile([16, B], F32, tag="acc", name="cm2_ps")

            # ---------------- conv1
            for hc in range(8):
                c, half = hc // 2, hc % 2
                gt = gtiles[hc]
                if hc < 6:
                    gtiles.append(load_g((hc + 2) // 2, (hc + 2) % 2))
                if hc == 5:
                    cm2_block(0, cm2_ps)   # H1[0] masked during hc4
                    stage2(0)
                for e in range(4 * half, 4 * half + 4):
                    pp = ppool.tile([128, 2 * B], F16, tag="pp", name="pp")
                    for oy2 in range(2):
                        oy = 4 * e + 2 * oy2
                        yoff = 16 * half  # tile col = y - yoff
                        ps = cps.tile([128, 2 * B], F32, tag="cps", name="cps")
                        gv = gt[:].rearrange("p (y b) -> p y b", y=20)
                        for sub in range(2):
                            nc.tensor.matmul(
                                ps[:, sub * B : (sub + 1) * B],
                                L1A[:],
                                gv[:, oy + sub - yoff, :],
                                start=True,
                                stop=False,
                            )
                            nc.tensor.matmul(
                                ps[:, sub * B : (sub + 1) * B],
                                L1B[:],
                                gv[0:72, oy + sub + 3 - yoff, :],
                                start=False,
                                stop=True,
                            )
                        a = apool.tile([128, 2 * B], F16, tag="act", name="act")
                        nc.scalar.activation(a[:], ps[:], AF.Relu, bias=B1[:])
                        nc.vector.tensor_tensor(
                            pp[:, oy2 * B : (oy2 + 1) * B],
                            a[:, 0:B],
                            a[:, B : 2 * B],
                            ALU.max,
                        )
                    mv = mvpool.tile([64, 2 * B], F16, tag="mv", name="mv")
                    nc.gpsimd.dma_start(mv[:], pp[64:128, :])
                    flush_fold()
                    # chunk c covers px 4c..4c+3 -> H1[c//2] at offset (c%2)*64
                    pend[0] = (
                        H1[c // 2][
                            (c % 2) * 64 : (c % 2) * 64 + 64,
                            2 * e * B : (2 * e + 2) * B,
                        ],
                        pp,
                        mv,
                    )
                    if hc == 4:
                        mask_h1_piece(0, e)
            flush_fold()
            for j in range(4):
                mask_h1_piece(1, j)
            stage2(1)

            # ---------------- conv2 (trailing ops of each d emitted one d later)
            cm3_ps = None
            mm2 = None
            qh2 = qc2 = None
            for d in range(4):
                nk = 96 if d in (0, 3) else 128
                rhs = r2[("c2", d)]

                def lhs2(ky, d=d):
                    if d == 0:
                        return L2E0[ky][:]
                    if d == 3:
                        return L2[ky][0:96, :]
                    return L2[ky][:]

                rv = rhs[0:nk, :].rearrange("p (y b) -> p y b", y=16)
                for oy2 in range(8):
                    ps = cps.tile([128, 2 * B], F32, tag="cps", name="cps")
                    for sub in range(2):
                        oy = 2 * oy2 + sub
                        kys = [k for k in range(5) if 0 <= oy + k - 2 < 16]
                        for j, ky in enumerate(kys):
                            nc.tensor.matmul(
                                ps[:, sub * B : (sub + 1) * B],
                                lhs2(ky),
                                rv[:, oy + ky - 2, :],
                                start=(j == 0),
                                stop=(j == len(kys) - 1),
                            )
                    a = apool.tile([128, 2 * B], F16, tag="act", name="act")
                    nc.scalar.activation(a[:], ps[:], AF.Relu, bias=B2[:])
                    if oy2 % 2 == 0:
                        pp = ppool.tile([128, 2 * B], F16, tag="pp", name="pp")
                    nc.vector.tensor_tensor(
                        pp[:, (oy2 % 2) * B : (oy2 % 2 + 1) * B],
                        a[:, 0:B],
                        a[:, B : 2 * B],
                        ALU.max,
                    )
                    if oy2 % 2 == 1:
                        mv = mvpool.tile([64, 2 * B], F16, tag="mv", name="mv")
                        nc.gpsimd.dma_start(mv[:], pp[64:128, :])
                        flush_fold()
                        j2 = oy2 // 2  # pooled-row pair index
                        pend[0] = (
                            H2[d // 2][
                                (d % 2) * 64 : (d % 2) * 64 + 64,
                                2 * j2 * B : (2 * j2 + 2) * B,
                            ],
                            pp,
                            mv,
                        )
                if d == 0:
                    cm2_block(1, cm2_ps)
                    stage2(2)
                elif d == 1:
                    qh2, qc2 = qchain_a(cm2_ps[:], 16, A2V, ON16, TC2, "2")
                    stage2(3)
                elif d == 2:
                    mm2 = qchain_b(qh2, qc2, KH2T, KH2CT, "2")
                    mask_h2(0, mm2)
                    cm3_ps = acc.tile([20, B], F32, tag="acc", name="cm3_ps")
                    cm3_block(0, cm3_ps)
                    stage3(0)
            flush_fold()
            mask_h2(1, mm2)
            stage3(1)

            # ---------------- conv3
            mm3 = None
            qh3 = qc3 = None
            fc_ps = None
            for d in range(4):
                nk = 80 if d in (0, 3) else 120
                rhs = r2[("c3", d)]

                def lhs3(ky, d=d):
                    if d == 0:
                        return L3E0[ky][:]
                    if d == 3:
                        return L3[ky][0:80, :]
                    return L3[ky][:]

                rv = rhs[0:nk, 0 : 8 * B].rearrange("p (y b) -> p y b", y=8)
                for oy2 in range(4):
                    ps = cps.tile([128, 2 * B], F32, tag="cps", name="cps")
                    for sub in range(2):
                        oy = 2 * oy2 + sub
                        kys = [k for k in range(5) if 0 <= oy + k - 2 < 8]
                        for j, ky in enumerate(kys):
                            nc.tensor.matmul(
                                ps[:, sub * B : (sub + 1) * B],
                                lhs3(ky),
                                rv[:, oy + ky - 2, :],
                                start=(j == 0),
                                stop=(j == len(kys) - 1),
                            )
                    a = apool.tile([128, 2 * B], F16, tag="act", name="act")
                    nc.scalar.activation(a[:], ps[:], AF.Relu, bias=B3[:])
                    if oy2 % 2 == 0:
                        pp = ppool.tile([128, 2 * B], F16, tag="pp", name="pp")
                    nc.vector.tensor_tensor(
                        pp[:, (oy2 % 2) * B : (oy2 % 2 + 1) * B],
                        a[:, 0:B],
                        a[:, B : 2 * B],
                        ALU.max,
                    )
                    if oy2 % 2 == 1:
                        mv = mvpool.tile([64, 2 * B], F16, tag="mv", name="mv")
                        nc.gpsimd.dma_start(mv[:], pp[64:128, :])
                        flush_fold()
                        j2 = oy2 // 2
                        pend[0] = (
                            H3[d][:, 2 * j2 * B : (2 * j2 + 2) * B],
                            pp,
                            mv,
                        )
                if d == 0:
                    cm3_block(1, cm3_ps)
                    qh3, qc3 = qchain_a(cm3_ps[:], 20, A3V, ON20, TC3, "3")
                    stage3(2)
                elif d == 1:
                    mm3 = qchain_b(qh3, qc3, KH3T, KH3CT, "3")
                    fc_ps = acc.tile([10, B], F32, tag="acc", name="fc_ps")
                    stage3(3)
                if d >= 1:
                    dm = d - 1  # mask + FC for the previous (complete) chunk
                    h3v = H3[dm][:].rearrange("p (y b) -> p y b", y=4)
                    mb = mm3[0:64, :].broadcast_to((64, B, 4)).rearrange("p b y -> p y b")
                    nc.vector.tensor_tensor(h3v, h3v, mb, ALU.mult)
                    for oyp in range(4):
                        nc.tensor.matmul(
                            fc_ps[:],
                            LO[:, (dm * 4 + oyp) * 10 : (dm * 4 + oyp) * 10 + 10],
                            H3[dm][:, oyp * B : (oyp + 1) * B],
                            start=(dm == 0 and oyp == 0),
                            stop=False,
                            skip_group_check=True,
                        )
            flush_fold()
            h3v = H3[3][:].rearrange("p (y b) -> p y b", y=4)
            mb = mm3[0:64, :].broadcast_to((64, B, 4)).rearrange("p b y -> p y b")
            nc.vector.tensor_tensor(h3v, h3v, mb, ALU.mult)
            for oyp in range(4):
                nc.tensor.matmul(
                    fc_ps[:],
                    LO[:, (3 * 4 + oyp) * 10 : (3 * 4 + oyp) * 10 + 10],
                    H3[3][:, oyp * B : (oyp + 1) * B],
                    start=False,
                    stop=(oyp == 3),
                    skip_group_check=True,
                )

            ob = qpool.tile([10, B], F32, tag="outb", name="outb")
            nc.scalar.activation(ob[:], fc_ps[:], AF.Identity, bias=BO[:])
            nc.sync.dma_start(out[:], ob[:])

    nc.compile()
    return nc


# ---------------------------------------------------------------- entry point
def kernel(**inputs) -> np.ndarray:
    in_maps = _host_prep(inputs)
    if "nc" not in _CACHED:
        _CACHED["nc"] = build_kernel()
    nc = _CACHED["nc"]
    res = run_bass_kernel_spmd(nc, in_maps, core_ids=list(range(NCORES)))
    outs = [res.results[i]["out"].T for i in range(NCORES)]
    return np.ascontiguousarray(np.concatenate(outs, axis=0)).astype(np.float32)


# revision 21
# speedup vs baseline: 1.1726x; 1.0734x over previous
"""ALSHConvNet on 8 TRN2 NeuronCores — pure data parallel (batch/8 per core).

Per core (512 samples):
- Convs as fp16 banded matmuls on TensorE, fp32 PSUM accumulation, M-order
  (parity, xpair, ch) with channel count padded to a power-of-two block so
  maxpool-x partners sit exactly 64 partitions apart (legal DVE offset).
- Weight-side hash bits AND the full layer-1 ALSH mask are computed on host;
  layer-2/3 query hashes run on DVE/ACT fully overlapped with conv matmuls.
- Mask applied once per layer on the pooled tensor (mask commutes with
  maxpool since it is a constant 0/1 per (sample, channel)).
- Maxpool: y-pairs = DVE max of the two halves of a 2-bank PSUM activation;
  x-pairs = SBUF->SBUF DMA of the upper partition half + DVE max.
- conv1 rhs = single 36-column im2col group; the ky=3,4 pass reuses the same
  SBUF data at a +3 column offset. y-edges of conv2/conv3 skip pad-ky
  matmuls; x-edges use K-trimmed weight tiles (no memsets).
- Host does layout + hashing of host-known quantities only: sharding,
  im2col, banded weights, masks, and the final [10,B] -> [B,10] transpose.
"""

import sys

for p in ("/opt/trn_rl_repo",):
    if p not in sys.path:
        sys.path.insert(0, p)

import numpy as np

import concourse.bass as bass  # noqa
import concourse.bacc as bacc
import concourse.mybir as mybir
import concourse.tile as tile
from concourse.bass_utils import run_bass_kernel_spmd

F32 = mybir.dt.float32
F16 = mybir.dt.float16
AF = mybir.ActivationFunctionType
ALU = mybir.AluOpType
AX = mybir.AxisListType

NCORES = 8
B = 512
R = 0.2
EPS = 1e-12
M_ALSH = 5

_CACHED = {}


# ---------------------------------------------------------------- host hashing
def _kernel_hash_bits(W, a, c):
    """Weight-side ALSH hash bits, fp32, mirroring reference.alsh_mask."""
    W = W.astype(np.float32)
    a = a.astype(np.float32)
    Cout = W.shape[0]
    Kf = W.reshape(Cout, -1)
    norms = np.linalg.norm(Kf, axis=1)
    Kn = Kf / (np.float32(norms.max()) + np.float32(EPS))
    n = np.linalg.norm(Kn, axis=1, keepdims=True).astype(np.float32)
    powers = np.concatenate(
        [n ** np.float32(2 ** (i + 1)) for i in range(M_ALSH)], axis=1
    ).astype(np.float32)
    P = np.concatenate([Kn, powers], axis=1)
    kh = np.mod(np.floor((P @ a + np.float32(c[0])) / np.float32(R)), 2.0)
    return kh.astype(np.float32)  # [Cout] in {0,1}


def _query_hash_bits_l1(x, a1, c1):
    """Per-sample layer-1 query hash bits, fp32, mirroring reference."""
    x = x.astype(np.float32)
    cm = x.mean(axis=(2, 3))                       # [B, 3]
    q = np.repeat(cm, 25, axis=1)                  # [B, 75]
    qn = q / (np.linalg.norm(q, axis=1, keepdims=True) + np.float32(EPS))
    Qv = np.concatenate(
        [qn, np.full((q.shape[0], M_ALSH), 0.5, np.float32)], axis=1
    )
    qh = np.mod(np.floor((Qv @ a1.astype(np.float32) + np.float32(c1[0])) / np.float32(R)), 2.0)
    return qh.astype(np.float32)  # [B] in {0,1}


# ---------------------------------------------------------------- host layout
def _band_lhsT1(W1):
    """conv1 lhsT: pass1 [108,128] (ky 0-2), pass2 [72,128] (ky 3-4).
    M-order: m = par*64 + oxp*16 + co, oxl = 2*oxp + par, wx = oxl + kx."""
    l0 = np.zeros((108, 128), np.float32)
    l1 = np.zeros((72, 128), np.float32)
    for par in range(2):
        for oxp in range(4):
            for co in range(16):
                m = par * 64 + oxp * 16 + co
                oxl = 2 * oxp + par
                for ky in range(5):
                    for ci in range(3):
                        for kx in range(5):
                            wx = oxl + kx
                            if ky < 3:
                                l0[ky * 36 + ci * 12 + wx, m] = W1[co, ci, ky, kx]
                            else:
                                l1[(ky - 3) * 36 + ci * 12 + wx, m] = W1[co, ci, ky, kx]
    return l0.astype(np.float16), l1.astype(np.float16)


def _band_lhsT2(W2):
    """conv2 lhsT per ky: [128 = wx*16+ci, 128 = par*64 + oxp*32 + co(pad32)]."""
    l = np.zeros((5, 128, 128), np.float32)
    for ky in range(5):
        for par in range(2):
            for oxp in range(2):
                for co in range(20):
                    m = par * 64 + oxp * 32 + co
                    oxl = 2 * oxp + par
                    for ci in range(16):
                        for kx in range(5):
                            l[ky, (oxl + kx) * 16 + ci, m] = W2[co, ci, ky, kx]
    return l.astype(np.float16)


def _band_lhsT3(W3):
    """conv3 lhsT per ky: [120 = wx*20+ci, 128 = par*64 + co(pad64)]."""
    l = np.zeros((5, 120, 128), np.float32)
    for ky in range(5):
        for par in range(2):
            for co in range(20):
                m = par * 64 + co
                for ci in range(20):
                    for kx in range(5):
                        l[ky, (par + kx) * 20 + ci, m] = W3[co, ci, ky, kx]
    return l.astype(np.float16)


def _fc_lhsT(Wo):
    """[64 = co(pad64), 160 = (d*4+oyp)*10 + o]; h flat idx = co*16 + oyp*4 + d."""
    l = np.zeros((64, 160), np.float32)
    for d in range(4):
        for oyp in range(4):
            for co in range(20):
                l[co, (d * 4 + oyp) * 10 : (d * 4 + oyp) * 10 + 10] = Wo[
                    :, co * 16 + oyp * 4 + d
                ]
    return l.astype(np.float16)


def _im2col1(xs):
    """g [4, 108, 36, B]: g[c][dy*36+ci*12+wx, y, b] = xpad[b, ci, y+dy, 8c+wx]."""
    xp = np.zeros((B, 3, 38, 36), np.float16)
    xp[:, :, 2:34, 2:34] = xs.astype(np.float16)
    g = np.empty((4, 108, 36, B), np.float16)
    for c in range(4):
        for dy in range(3):
            blk = xp[:, :, dy : dy + 36, 8 * c : 8 * c + 12]  # [B,3,36,12]
            g[c, dy * 36 : (dy + 1) * 36] = (
                blk.transpose(1, 3, 2, 0).reshape(36, 36, B)
            )
    return g


def _host_prep(inputs):
    x = inputs["x"].astype(np.float32)
    W1 = inputs["W1"].astype(np.float32)
    W2 = inputs["W2"].astype(np.float32)
    W3 = inputs["W3"].astype(np.float32)
    b1 = inputs["b1"].astype(np.float32)
    b2 = inputs["b2"].astype(np.float32)
    b3 = inputs["b3"].astype(np.float32)
    a1 = inputs["a1"].astype(np.float32)
    a2 = inputs["a2"].astype(np.float32)
    a3 = inputs["a3"].astype(np.float32)

    kh1 = _kernel_hash_bits(W1, a1, inputs["c1"])  # [16]
    kh2 = _kernel_hash_bits(W2, a2, inputs["c2"])  # [20]
    kh3 = _kernel_hash_bits(W3, a3, inputs["c3"])  # [20]
    qh1 = _query_hash_bits_l1(x, a1, inputs["c1"])  # [4096]
    m1 = (kh1[None, :] == qh1[:, None]).astype(np.float32)  # [4096, 16]

    l1a, l1b = _band_lhsT1(W1)
    l1bp = np.zeros((108, 128), np.float16)
    l1bp[0:72] = l1b
    l1b = l1bp
    l2 = _band_lhsT2(W2)
    l3 = _band_lhsT3(W3)

    def padco(b, n):
        o = np.zeros(n, np.float32)
        o[: b.shape[0]] = b
        return o

    b2p = padco(b2, 32)
    b3p = padco(b3, 64)
    kh2p = padco(kh2, 32)
    kh2cp = padco(1.0 - kh2, 32)
    kh3p = padco(kh3, 64)
    kh3cp = padco(1.0 - kh3, 64)

    shared = {
        "l1a": l1a,
        "l1b": l1b,
        "l2": l2,                      # [5,128,128]
        "l2e0": np.ascontiguousarray(l2[:, 32:128, :]),  # [5,96,128]
        "l3": l3,                      # [5,120,128]
        "l3e0": np.ascontiguousarray(l3[:, 40:120, :]),  # [5,80,128]
        "lo": _fc_lhsT(inputs["Wo"].astype(np.float32)),  # [64,160]
        "s2b": np.tile(np.eye(16, dtype=np.float16), (8, 1)),  # [128,16]
        "s3b": np.concatenate(
            [
                np.concatenate(
                    [np.eye(20, dtype=np.float16), np.zeros((12, 20), np.float16)],
                    axis=0,
                )
                for _ in range(4)
            ],
            axis=0,
        ),  # [128,20]
        "b1m": np.tile(b1, 8).reshape(128, 1),
        "b2m": np.tile(b2p, 4).reshape(128, 1),
        "b3m": np.tile(b3p, 2).reshape(128, 1),
        "bo": inputs["bo"].reshape(10, 1).astype(np.float32),
        "a2v": a2[:400].reshape(16, 25).sum(axis=1).reshape(16, 1),
        "a3v": a3[:500].reshape(20, 25).sum(axis=1).reshape(20, 1),
        "ones16": np.ones((16, 1), np.float32),
        "ones20": np.ones((20, 1), np.float32),
        "tc2": np.array(
            [[0.5 * a2[400:].sum() + inputs["c2"].astype(np.float32)[0]]], np.float32
        ),
        "tc3": np.array(
            [[0.5 * a3[500:].sum() + inputs["c3"].astype(np.float32)[0]]], np.float32
        ),
        "kh2t": np.tile(kh2p, 4).reshape(1, 128),
        "kh2ct": np.tile(kh2cp, 4).reshape(1, 128),
        "kh3t": np.tile(kh3p, 2).reshape(1, 128),
        "kh3ct": np.tile(kh3cp, 2).reshape(1, 128),
    }
    in_maps = []
    for i in range(NCORES):
        xs = x[i * B : (i + 1) * B]
        m = dict(shared)
        m["g"] = _im2col1(xs)
        # [128 = px8*16+ci, B] mask for H1 tiles (same pattern both tiles)
        m["mm1h"] = np.tile(m1[i * B : (i + 1) * B].T, (8, 1)).astype(np.float16)
        in_maps.append(m)
    return in_maps


# ---------------------------------------------------------------- device build
def _parity_ge1(nc, pool, t_ap, C, outtile):
    """outtile = (floor(t) mod 2) as 0/1 via fp32 magic rounding."""
    MAGIC = 12582912.0  # 1.5 * 2^23
    a = pool.tile([C, t_ap.shape[1]], F32, tag="par_a", name="par_a")
    nc.vector.tensor_scalar(a[:], t_ap, 0.5, -0.5, ALU.mult, ALU.add)
    nc.vector.tensor_scalar_add(a[:], a[:], MAGIC)
    nc.vector.tensor_scalar_add(a[:], a[:], -MAGIC)  # a = floor(t/2)
    u = pool.tile([C, t_ap.shape[1]], F32, tag="par_u", name="par_u")
    nc.vector.scalar_tensor_tensor(u[:], a[:], -2.0, t_ap, ALU.mult, ALU.add)
    nc.vector.tensor_scalar(outtile, u[:], 1.0, None, ALU.is_ge)


def build_kernel():
    nc = bacc.Bacc(None, target_bir_lowering=False, debug=False)

    def din(name, shape, dtype=F32):
        return nc.dram_tensor(name, list(shape), dtype, kind="ExternalInput").ap()

    g_in = din("g", (4, 108, 36, B), F16)
    l1a_in = din("l1a", (108, 128), F16)
    l1b_in = din("l1b", (108, 128), F16)
    l2_in = din("l2", (5, 128, 128), F16)
    l2e0_in = din("l2e0", (5, 96, 128), F16)
    l3_in = din("l3", (5, 120, 128), F16)
    l3e0_in = din("l3e0", (5, 80, 128), F16)
    lo_in = din("lo", (64, 160), F16)
    s2b_in = din("s2b", (128, 16), F16)
    s3b_in = din("s3b", (128, 20), F16)
    b1m_in = din("b1m", (128, 1))
    b2m_in = din("b2m", (128, 1))
    b3m_in = din("b3m", (128, 1))
    bo_in = din("bo", (10, 1))
    a2v_in = din("a2v", (16, 1))
    a3v_in = din("a3v", (20, 1))
    ones16_in = din("ones16", (16, 1))
    ones20_in = din("ones20", (20, 1))
    tc2_in = din("tc2", (1, 1))
    tc3_in = din("tc3", (1, 1))
    kh2t_in = din("kh2t", (1, 128))
    kh2ct_in = din("kh2ct", (1, 128))
    kh3t_in = din("kh3t", (1, 128))
    kh3ct_in = din("kh3ct", (1, 128))
    mm1h_in = din("mm1h", (128, B), F16)
    out = nc.dram_tensor("out", [10, B], F32, kind="ExternalOutput").ap()

    with tile.TileContext(nc) as tc:
        with (
            tc.tile_pool(name="const", bufs=1) as cpool,
            tc.tile_pool(name="g", bufs=3) as gpool,
            tc.tile_pool(name="h", bufs=1) as hpool,
            tc.tile_pool(name="rhs", bufs=2) as rpool,
            tc.tile_pool(name="apool", bufs=3) as apool,
            tc.tile_pool(name="ppool", bufs=3) as ppool,
            tc.tile_pool(name="mvpool", bufs=2) as mvpool,
            tc.tile_pool(name="q", bufs=1) as qpool,
            tc.tile_pool(name="cps", bufs=3, space="PSUM") as cps,
            tc.tile_pool(name="acc", bufs=2, space="PSUM") as acc,
        ):
            def load_const(ap, dtype, tag, eng=None):
                t = cpool.tile(list(ap.shape), dtype, tag=tag, name=tag)
                (eng or nc.scalar).dma_start(t[:], ap[:])
                return t

            L1A = load_const(l1a_in, F16, "l1a")
            L1B = load_const(l1b_in, F16, "l1b")
            B1 = load_const(b1m_in, F32, "b1m")

            def load_g(c, half):
                """Half-chunk: cols y in [0,20) (half 0) or [16,36) (half 1)."""
                t = gpool.tile([108, 20 * B], F16, tag="g", name=f"g{c}_{half}")
                tv = t[:].rearrange("p (y b) -> p y b", y=20)
                yb = 16 * half
                for y0, y1 in ((0, 8), (8, 14), (14, 20)):
                    nc.sync.dma_start(tv[:, y0:y1, :], g_in[c, :, yb + y0 : yb + y1, :])
                return t

            gtiles = [load_g(0, 0), load_g(0, 1)]

            MM1H = load_const(mm1h_in, F16, "mm1h")
            S2B = load_const(s2b_in, F16, "s2b")
            S3B = load_const(s3b_in, F16, "s3b")
            B2 = load_const(b2m_in, F32, "b2m")
            B3 = load_const(b3m_in, F32, "b3m")
            BO = load_const(bo_in, F32, "bo")
            A2V = load_const(a2v_in, F32, "a2v")
            A3V = load_const(a3v_in, F32, "a3v")
            ON16 = load_const(ones16_in, F32, "ones16")
            ON20 = load_const(ones20_in, F32, "ones20")
            TC2 = load_const(tc2_in, F32, "tc2")
            TC3 = load_const(tc3_in, F32, "tc3")
            KH2T = load_const(kh2t_in, F32, "kh2t")
            KH2CT = load_const(kh2ct_in, F32, "kh2ct")
            KH3T = load_const(kh3t_in, F32, "kh3t")
            KH3CT = load_const(kh3ct_in, F32, "kh3ct")
            L2 = [load_const(l2_in[k], F16, f"l2_{k}", nc.gpsimd) for k in range(5)]
            L2E0 = [load_const(l2e0_in[k], F16, f"l2e0_{k}", nc.gpsimd) for k in range(5)]
            L3 = [load_const(l3_in[k], F16, f"l3_{k}", nc.gpsimd) for k in range(5)]
            L3E0 = [load_const(l3e0_in[k], F16, f"l3e0_{k}", nc.gpsimd) for k in range(5)]
            LO = load_const(lo_in, F16, "lo", nc.gpsimd)

            H1 = [
                hpool.tile([128, 16 * B], F16, tag=f"h1_{i}", name=f"h1_{i}")
                for i in range(2)
            ]
            H2 = [
                hpool.tile([128, 8 * B], F16, tag=f"h2_{i}", name=f"h2_{i}")
                for i in range(2)
            ]
            H3 = [
                hpool.tile([64, 4 * B], F16, tag=f"h3_{d}", name=f"h3_{d}")
                for d in range(4)
            ]

            r2 = {}

            def stage2(d):
                rhs = rpool.tile([128, 16 * B], F16, tag="rhs2", name=f"rhs2_{d}")
                if d == 0:
                    nc.sync.dma_start(rhs[0:96, :], H1[0][0:96, :])
                elif d == 1:
                    nc.sync.dma_start(rhs[0:96, :], H1[0][32:128, :])
                    nc.sync.dma_start(rhs[96:128, :], H1[1][0:32, :])
                elif d == 2:
                    nc.sync.dma_start(rhs[0:32, :], H1[0][96:128, :])
                    nc.sync.dma_start(rhs[32:128, :], H1[1][0:96, :])
                else:
                    nc.sync.dma_start(rhs[0:96, :], H1[1][32:128, :])
                r2[("c2", d)] = rhs

            def stage3(d):
                rhs = rpool.tile([128, 16 * B], F16, tag="rhs2", name=f"rhs3_{d}")
                wxs = range(2, 6) if d == 0 else (range(0, 4) if d == 3 else range(6))
                for r, wx in enumerate(wxs):
                    px3 = 2 * d - 2 + wx
                    src = H2[px3 // 4]
                    p0 = (px3 % 4) * 32
                    nc.sync.dma_start(
                        rhs[r * 20 : (r + 1) * 20, 0 : 8 * B], src[p0 : p0 + 20, :]
                    )
                r2[("c3", d)] = rhs

            def mask_h1_piece(i, j):
                """Multiply mask into H1[i] cols [4j,4j+4) (4 of 16 oy)."""
                h1v = H1[i][:, 4 * j * B : (4 * j + 4) * B].rearrange(
                    "p (y b) -> p y b", y=4
                )
                mb = MM1H[:].broadcast_to((128, B, 4)).rearrange("p b y -> p y b")
                nc.vector.tensor_tensor(h1v, h1v, mb, ALU.mult)

            def cm2_block(i, cm2_ps):
                for oy in range(16):
                    nc.tensor.matmul(
                        cm2_ps[:],
                        S2B[:],
                        H1[i][:, oy * B : (oy + 1) * B],
                        start=(i == 0 and oy == 0),
                        stop=(i == 1 and oy == 15),
                        skip_group_check=True,
                    )

            def mask_h2(i, mm2):
                for j in range(2):
                    h2v = H2[i][:, 4 * j * B : (4 * j + 4) * B].rearrange(
                        "p (y b) -> p y b", y=4
                    )
                    mb = mm2[:].broadcast_to((128, B, 4)).rearrange("p b y -> p y b")
                    nc.vector.tensor_tensor(h2v, h2v, mb, ALU.mult)

            def cm3_block(i, cm3_ps):
                for oy in range(8):
                    nc.tensor.matmul(
                        cm3_ps[:],
                        S3B[:],
                        H2[i][:, oy * B : (oy + 1) * B],
                        start=(i == 0 and oy == 0),
                        stop=(i == 1 and oy == 7),
                        skip_group_check=True,
                    )

            def qchain_a(cmps_ap, C, AV, ONESC, TC, lname):
                cmsb = qpool.tile([C, B], F32, tag="q_cmsb", name=f"cmsb{lname}")
                nc.scalar.activation(cmsb[:], cmps_ap, AF.Identity)
                sq = qpool.tile([C, B], F32, tag="q_sq", name=f"sq{lname}")
                nc.scalar.activation(sq[:], cmps_ap, AF.Square)
                num_ps = acc.tile([1, B], F32, tag="acc", name=f"num{lname}")
                nc.tensor.matmul(num_ps[:], AV[:, 0:1], cmsb[:], start=True, stop=True)
                den_ps = acc.tile([1, B], F32, tag="acc", name=f"den{lname}")
                nc.tensor.matmul(den_ps[:], ONESC[:, 0:1], sq[:], start=True, stop=True)
                den = qpool.tile([1, B], F32, tag="q_den", name=f"den{lname}")
                nc.scalar.activation(den[:], den_ps[:], AF.Sqrt, scale=25.0)
                nc.vector.tensor_scalar_add(den[:], den[:], EPS)
                rden = qpool.tile([1, B], F32, tag="q_rden", name=f"rden{lname}")
                nc.vector.reciprocal(rden[:], den[:])
                nums = qpool.tile([1, B], F32, tag="q_nums", name=f"nums{lname}")
                nc.scalar.activation(nums[:], num_ps[:], AF.Identity)
                qv = qpool.tile([1, B], F32, tag="q_qv", name=f"qv{lname}")
                nc.vector.tensor_tensor(qv[:], nums[:], rden[:], ALU.mult)
                nc.vector.tensor_scalar(qv[:], qv[:], TC[0:1, 0:1], 1.0 / R, ALU.add, ALU.mult)
                qh = qpool.tile([1, B], F32, tag="q_qh", name=f"qh{lname}")
                _parity_ge1(nc, qpool, qv[:], 1, qh[:])
                qc = qpool.tile([1, B], F32, tag="q_qc", name=f"qc{lname}")
                nc.vector.tensor_scalar(qc[:], qh[:], -1.0, 1.0, ALU.mult, ALU.add)
                return qh, qc

            def qchain_b(qh, qc, KHT, KHCT, lname):
                map_ps = acc.tile([128, B], F32, tag="acc", name=f"map{lname}")
                nc.tensor.matmul(map_ps[:], KHT[0:1, :], qh[:], start=True, stop=False)
                nc.tensor.matmul(map_ps[:], KHCT[0:1, :], qc[:], start=False, stop=True)
                mm = hpool.tile([128, B], F16, tag=f"mm{lname}", name=f"mm{lname}")
                nc.scalar.activation(mm[:], map_ps[:], AF.Identity)
                return mm

            pend = [None]  # delayed x-fold: (dst_ap, pp, mv)

            def flush_fold():
                if pend[0] is not None:
                    dst, fpp, fmv = pend[0]
                    nc.vector.tensor_tensor(dst, fpp[0:64, :], fmv[:], ALU.max)
                    pend[0] = None

            cm2_ps = acc.tile([16, B], F32, tag="acc", name="cm2_ps")

            # ---------------- conv1
            for hc in range(8):
                c, half = hc // 2, hc % 2
                gt = gtiles[hc]
                if hc < 6:
                    gtiles.append(load_g((hc + 2) // 2, (hc + 2) % 2))
                if hc == 5:
                    cm2_block(0, cm2_ps)   # H1[0] masked during hc4
                    stage2(0)
                for e in range(4 * half, 4 * half + 4):
                    pp = ppool.tile([128, 2 * B], F16, tag="pp", name="pp")
                    for oy2 in range(2):
                        oy = 4 * e + 2 * oy2
                        yoff = 16 * half  # tile col = y - yoff
                        ps = cps.tile([128, 2 * B], F32, tag="cps", name="cps")
                        gv = gt[:].rearrange("p (y b) -> p y b", y=20)
                        for sub in range(2):
                            nc.tensor.matmul(
                                ps[:, sub * B : (sub + 1) * B],
                                L1A[:],
                                gv[:, oy + sub - yoff, :],
                                start=True,
                                stop=False,
                            )
                            nc.tensor.matmul(
                                ps[:, sub * B : (sub + 1) * B],
                                L1B[:],
                                gv[:, oy + sub + 3 - yoff, :],
                                start=False,
                                stop=True,
                            )
                        a = apool.tile([128, 2 * B], F16, tag="act", name="act")
                        nc.scalar.activation(a[:], ps[:], AF.Relu, bias=B1[:])
                        nc.vector.tensor_tensor(
                            pp[:, oy2 * B : (oy2 + 1) * B],
                            a[:, 0:B],
                            a[:, B : 2 * B],
                            ALU.max,
                        )
                    mv = mvpool.tile([64, 2 * B], F16, tag="mv", name="mv")
                    nc.gpsimd.dma_start(mv[:], pp[64:128, :])
                    flush_fold()
                    # chunk c covers px 4c..4c+3 -> H1[c//2] at offset (c%2)*64
                    pend[0] = (
                        H1[c // 2][
                            (c % 2) * 64 : (c % 2) * 64 + 64,
                            2 * e * B : (2 * e + 2) * B,
                        ],
                        pp,
                        mv,
                    )
                    if hc == 4:
                        mask_h1_piece(0, e)
            flush_fold()
            for j in range(4):
                mask_h1_piece(1, j)
            stage2(1)

            # ---------------- conv2 (trailing ops of each d emitted one d later)
            cm3_ps = None
            mm2 = None
            qh2 = qc2 = None
            for d in range(4):
                nk = 96 if d in (0, 3) else 128
                rhs = r2[("c2", d)]

                def lhs2(ky, d=d):
                    if d == 0:
                        return L2E0[ky][:]
                    if d == 3:
                        return L2[ky][0:96, :]
                    return L2[ky][:]

                rv = rhs[0:nk, :].rearrange("p (y b) -> p y b", y=16)
                for oy2 in range(8):
                    ps = cps.tile([128, 2 * B], F32, tag="cps", name="cps")
                    for sub in range(2):
                        oy = 2 * oy2 + sub
                        kys = [k for k in range(5) if 0 <= oy + k - 2 < 16]
                        for j, ky in enumerate(kys):
                            nc.tensor.matmul(
                                ps[:, sub * B : (sub + 1) * B],
                                lhs2(ky),
                                rv[:, oy + ky - 2, :],
                                start=(j == 0),
                                stop=(j == len(kys) - 1),
                            )
                    a = apool.tile([128, 2 * B], F16, tag="act", name="act")
                    nc.scalar.activation(a[:], ps[:], AF.Relu, bias=B2[:])
                    if oy2 % 2 == 0:
                        pp = ppool.tile([128, 2 * B], F16, tag="pp", name="pp")
                    nc.vector.tensor_tensor(
                        pp[:, (oy2 % 2) * B : (oy2 % 2 + 1) * B],
                        a[:, 0:B],
                        a[:, B : 2 * B],
                        ALU.max,
                    )
                    if oy2 % 2 == 1:
                        mv = mvpool.tile([64, 2 * B], F16, tag="mv", name="mv")
                        nc.gpsimd.dma_start(mv[:], pp[64:128, :])
                        flush_fold()
                        j2 = oy2 // 2  # pooled-row pair index
                        pend[0] = (
                            H2[d // 2][
                                (d % 2) * 64 : (d % 2) * 64 + 64,
                                2 * j2 * B : (2 * j2 + 2) * B,
                            ],
                            pp,
                            mv,
                        )
                if d == 0:
                    cm2_block(1, cm2_ps)
                    stage2(2)
                elif d == 1:
                    qh2, qc2 = qchain_a(cm2_ps[:], 16, A2V, ON16, TC2, "2")
                    stage2(3)
                elif d == 2:
                    mm2 = qchain_b(qh2, qc2, KH2T, KH2CT, "2")
                    mask_h2(0, mm2)
                    cm3_ps = acc.tile([20, B], F32, tag="acc", name="cm3_ps")
                    cm3_block(0, cm3_ps)
                    stage3(0)
            flush_fold()
            mask_h2(1, mm2)
            stage3(1)

            # ---------------- conv3
            mm3 = None
            qh3 = qc3 = None
            fc_ps = None
            for d in range(4):
                nk = 80 if d in (0, 3) else 120
                rhs = r2[("c3", d)]

                def lhs3(ky, d=d):
                    if d == 0:
                        return L3E0[ky][:]
                    if d == 3:
                        return L3[ky][0:80, :]
                    return L3[ky][:]

                rv = rhs[0:nk, 0 : 8 * B].rearrange("p (y b) -> p y b", y=8)
                for oy2 in range(4):
                    ps = cps.tile([128, 2 * B], F32, tag="cps", name="cps")
                    for sub in range(2):
                        oy = 2 * oy2 + sub
                        kys = [k for k in range(5) if 0 <= oy + k - 2 < 8]
                        for j, ky in enumerate(kys):
                            nc.tensor.matmul(
                                ps[:, sub * B : (sub + 1) * B],
                                lhs3(ky),
                                rv[:, oy + ky - 2, :],
                                start=(j == 0),
                                stop=(j == len(kys) - 1),
                            )
                    a = apool.tile([128, 2 * B], F16, tag="act", name="act")
                    nc.scalar.activation(a[:], ps[:], AF.Relu, bias=B3[:])
                    if oy2 % 2 == 0:
                        pp = ppool.tile([128, 2 * B], F16, tag="pp", name="pp")
                    nc.vector.tensor_tensor(
                        pp[:, (oy2 % 2) * B : (oy2 % 2 + 1) * B],
                        a[:, 0:B],
                        a[:, B : 2 * B],
                        ALU.max,
                    )
                    if oy2 % 2 == 1:
                        mv = mvpool.tile([64, 2 * B], F16, tag="mv", name="mv")
                        nc.gpsimd.dma_start(mv[:], pp[64:128, :])
                        flush_fold()
                        j2 = oy2 // 2
                        pend[0] = (
                            H3[d][:, 2 * j2 * B : (2 * j2 + 2) * B],
                            pp,
                            mv,
                        )
                if d == 0:
                    cm3_block(1, cm3_ps)
                    qh3, qc3 = qchain_a(cm3_ps[:], 20, A3V, ON20, TC3, "3")
                    stage3(2)
                elif d == 1:
                    mm3 = qchain_b(qh3, qc3, KH3T, KH3CT, "3")
                    fc_ps = acc.tile([10, B], F32, tag="acc", name="fc_ps")
                    stage3(3)
                if d >= 1:
                    dm = d - 1  # mask + FC for the previous (complete) chunk
                    h3v = H3[dm][:].rearrange("p (y b) -> p y b", y=4)
                    mb = mm3[0:64, :].broadcast_to((64, B, 4)).rearrange("p b y -> p y b")
                    nc.vector.tensor_tensor(h3v, h3v, mb, ALU.mult)
                    for oyp in range(4):
                        nc.tensor.matmul(
                            fc_ps[:],
                            LO[:, (dm * 4 + oyp) * 10 : (dm * 4 + oyp) * 10 + 10],
                            H3[dm][:, oyp * B : (oyp + 1) * B],
                            start=(dm == 0 and oyp == 0),
                            stop=False,
                            skip_group_check=True,
                        )
            flush_fold()
            h3v = H3[3][:].rearrange("p (y b) -> p y b", y=4)
            mb = mm3[0:64, :].broadcast_to((64, B, 4)).rearrange("p b y -> p y b")
            nc.vector.tensor_tensor(h3v, h3v, mb, ALU.mult)
            for oyp in range(4):
                nc.tensor.matmul(
                    fc_ps[:],
                    LO[:, (3 * 4 + oyp) * 10 : (3 * 4 + oyp) * 10 + 10],
                    H3[3][:, oyp * B : (oyp + 1) * B],
                    start=False,
                    stop=(oyp == 3),
                    skip_group_check=True,
                )

            ob = qpool.tile([10, B], F32, tag="outb", name="outb")
            nc.scalar.activation(ob[:], fc_ps[:], AF.Identity, bias=BO[:])
            nc.sync.dma_start(out[:], ob[:])

    nc.compile()
    return nc


# ---------------------------------------------------------------- entry point
def kernel(**inputs) -> np.ndarray:
    in_maps = _host_prep(inputs)
    if "nc" not in _CACHED:
        _CACHED["nc"] = build_kernel()
    nc = _CACHED["nc"]
    res = run_bass_kernel_spmd(nc, in_maps, core_ids=list(range(NCORES)))
    outs = [res.results[i]["out"].T for i in range(NCORES)]
    return np.ascontiguousarray(np.concatenate(outs, axis=0)).astype(np.float32)


# revision 43
# speedup vs baseline: 1.5105x; 1.2882x over previous
"""ALSHConvNet on 8 TRN2 NeuronCores — pure data parallel (batch/8 per core).

Per core (512 samples):
- Convs as fp16 banded matmuls on TensorE, fp32 PSUM accumulation, M-order
  (parity, xpair, ch) with channel count padded to a power-of-two block so
  maxpool-x partners sit exactly 64 partitions apart (legal DVE offset).
- Weight-side hash bits AND the full layer-1 ALSH mask are computed on host;
  layer-2/3 query hashes run on DVE/ACT fully overlapped with conv matmuls.
- Mask applied once per layer on the pooled tensor (mask commutes with
  maxpool since it is a constant 0/1 per (sample, channel)).
- Maxpool: y-pairs = DVE max of the two halves of a 2-bank PSUM activation;
  x-pairs = SBUF->SBUF DMA of the upper partition half + DVE max.
- conv1 rhs = single 36-column im2col group; the ky=3,4 pass reuses the same
  SBUF data at a +3 column offset. y-edges of conv2/conv3 skip pad-ky
  matmuls; x-edges use K-trimmed weight tiles (no memsets).
- Host does layout + hashing of host-known quantities only: sharding,
  im2col, banded weights, masks, and the final [10,B] -> [B,10] transpose.
"""

import sys

for p in ("/opt/trn_rl_repo",):
    if p not in sys.path:
        sys.path.insert(0, p)

import numpy as np

import concourse.bass as bass  # noqa
import concourse.bacc as bacc
import concourse.mybir as mybir
import concourse.tile as tile
from concourse.bass_utils import run_bass_kernel_spmd

F32 = mybir.dt.float32
F16 = mybir.dt.float16
AF = mybir.ActivationFunctionType
ALU = mybir.AluOpType
AX = mybir.AxisListType

NCORES = 8
B = 512
R = 0.2
EPS = 1e-12
M_ALSH = 5

_CACHED = {}


# ---------------------------------------------------------------- host hashing
def _kernel_hash_bits(W, a, c):
    """Weight-side ALSH hash bits, fp32, mirroring reference.alsh_mask."""
    W = W.astype(np.float32)
    a = a.astype(np.float32)
    Cout = W.shape[0]
    Kf = W.reshape(Cout, -1)
    norms = np.linalg.norm(Kf, axis=1)
    Kn = Kf / (np.float32(norms.max()) + np.float32(EPS))
    n = np.linalg.norm(Kn, axis=1, keepdims=True).astype(np.float32)
    powers = np.concatenate(
        [n ** np.float32(2 ** (i + 1)) for i in range(M_ALSH)], axis=1
    ).astype(np.float32)
    P = np.concatenate([Kn, powers], axis=1)
    kh = np.mod(np.floor((P @ a + np.float32(c[0])) / np.float32(R)), 2.0)
    return kh.astype(np.float32)  # [Cout] in {0,1}


def _query_hash_bits_l1(x, a1, c1):
    """Per-sample layer-1 query hash bits, fp32, mirroring reference."""
    x = x.astype(np.float32)
    cm = x.mean(axis=(2, 3))                       # [B, 3]
    q = np.repeat(cm, 25, axis=1)                  # [B, 75]
    qn = q / (np.linalg.norm(q, axis=1, keepdims=True) + np.float32(EPS))
    Qv = np.concatenate(
        [qn, np.full((q.shape[0], M_ALSH), 0.5, np.float32)], axis=1
    )
    qh = np.mod(np.floor((Qv @ a1.astype(np.float32) + np.float32(c1[0])) / np.float32(R)), 2.0)
    return qh.astype(np.float32)  # [B] in {0,1}


# ---------------------------------------------------------------- host layout
def _band_lhsT1(W1):
    """conv1 lhsT: pass1 [108,128] (ky 0-2), pass2 [72,128] (ky 3-4).
    M-order: m = par*64 + oxp*16 + co, oxl = 2*oxp + par, wx = oxl + kx."""
    l0 = np.zeros((108, 128), np.float32)
    l1 = np.zeros((72, 128), np.float32)
    for par in range(2):
        for oxp in range(4):
            for co in range(16):
                m = par * 64 + oxp * 16 + co
                oxl = 2 * oxp + par
                for ky in range(5):
                    for ci in range(3):
                        for kx in range(5):
                            wx = oxl + kx
                            if ky < 3:
                                l0[ky * 36 + ci * 12 + wx, m] = W1[co, ci, ky, kx]
                            else:
                                l1[(ky - 3) * 36 + ci * 12 + wx, m] = W1[co, ci, ky, kx]
    return l0.astype(np.float16), l1.astype(np.float16)


def _band_lhsT2(W2):
    """conv2 lhsT per ky: [128 = wx*16+ci, 128 = par*64 + oxp*32 + co(pad32)]."""
    l = np.zeros((5, 128, 128), np.float32)
    for ky in range(5):
        for par in range(2):
            for oxp in range(2):
                for co in range(20):
                    m = par * 64 + oxp * 32 + co
                    oxl = 2 * oxp + par
                    for ci in range(16):
                        for kx in range(5):
                            l[ky, (oxl + kx) * 16 + ci, m] = W2[co, ci, ky, kx]
    return l.astype(np.float16)


def _band_lhsT3(W3):
    """conv3 lhsT per ky: [120 = wx*20+ci, 128 = par*64 + co(pad64)]."""
    l = np.zeros((5, 120, 128), np.float32)
    for ky in range(5):
        for par in range(2):
            for co in range(20):
                m = par * 64 + co
                for ci in range(20):
                    for kx in range(5):
                        l[ky, (par + kx) * 20 + ci, m] = W3[co, ci, ky, kx]
    return l.astype(np.float16)


def _fc_lhsT(Wo):
    """[64 = co(pad64), 160 = (d*4+oyp)*10 + o]; h flat idx = co*16 + oyp*4 + d."""
    l = np.zeros((64, 160), np.float32)
    for d in range(4):
        for oyp in range(4):
            for co in range(20):
                l[co, (d * 4 + oyp) * 10 : (d * 4 + oyp) * 10 + 10] = Wo[
                    :, co * 16 + oyp * 4 + d
                ]
    return l.astype(np.float16)


def _im2col1(xs):
    """g [4, 108, 36, B]: g[c][dy*36+ci*12+wx, y, b] = xpad[b, ci, y+dy, 8c+wx]."""
    xp = np.zeros((B, 3, 38, 36), np.float16)
    xp[:, :, 2:34, 2:34] = xs.astype(np.float16)
    g = np.zeros((4, 128, 36, B), np.float16)
    for c in range(4):
        for dy in range(3):
            blk = xp[:, :, dy : dy + 36, 8 * c : 8 * c + 12]  # [B,3,36,12]
            g[c, dy * 36 : (dy + 1) * 36] = (
                blk.transpose(1, 3, 2, 0).reshape(36, 36, B)
            )
    return g


def _padk(l):
    """Pad lhsT stack [5, k, 128] to [5, 128, 128] with zero rows."""
    o = np.zeros((5, 128, 128), np.float16)
    o[:, : l.shape[1], :] = l
    return o


def _host_prep(inputs):
    x = inputs["x"].astype(np.float32)
    W1 = inputs["W1"].astype(np.float32)
    W2 = inputs["W2"].astype(np.float32)
    W3 = inputs["W3"].astype(np.float32)
    b1 = inputs["b1"].astype(np.float32)
    b2 = inputs["b2"].astype(np.float32)
    b3 = inputs["b3"].astype(np.float32)
    a1 = inputs["a1"].astype(np.float32)
    a2 = inputs["a2"].astype(np.float32)
    a3 = inputs["a3"].astype(np.float32)

    kh1 = _kernel_hash_bits(W1, a1, inputs["c1"])  # [16]
    kh2 = _kernel_hash_bits(W2, a2, inputs["c2"])  # [20]
    kh3 = _kernel_hash_bits(W3, a3, inputs["c3"])  # [20]
    qh1 = _query_hash_bits_l1(x, a1, inputs["c1"])  # [4096]
    m1 = (kh1[None, :] == qh1[:, None]).astype(np.float32)  # [4096, 16]

    l1a, l1b = _band_lhsT1(W1)
    l1ap = np.zeros((128, 128), np.float16)
    l1ap[0:108] = l1a
    l1a = l1ap
    l1bp = np.zeros((128, 128), np.float16)
    l1bp[0:72] = l1b
    l1b = l1bp
    l2 = _band_lhsT2(W2)
    l3 = _band_lhsT3(W3)

    def padco(b, n):
        o = np.zeros(n, np.float32)
        o[: b.shape[0]] = b
        return o

    b2p = padco(b2, 32)
    b3p = padco(b3, 64)
    kh2p = padco(kh2, 32)
    kh2cp = padco(1.0 - kh2, 32)
    kh3p = padco(kh3, 64)
    kh3cp = padco(1.0 - kh3, 64)

    shared = {
        "l1a": l1a,
        "l1b": l1b,
        "l2": l2,                      # [5,128,128]
        "l2e0": _padk(l2[:, 32:128, :]),   # [5,128,128] rows 96+ zero
        "l2e3": _padk(l2[:, 0:96, :]),     # [5,128,128]
        "l3": _padk(l3),                   # [5,128,128] rows 120+ zero
        "l3e0": _padk(l3[:, 40:120, :]),   # [5,128,128]
        "l3e3": _padk(l3[:, 0:80, :]),     # [5,128,128]
        "lo": _fc_lhsT(inputs["Wo"].astype(np.float32)),  # [64,160]
        "s2b": np.tile(np.eye(16, dtype=np.float16), (8, 1)),  # [128,16]
        "s3b": np.concatenate(
            [
                np.concatenate(
                    [np.eye(20, dtype=np.float16), np.zeros((12, 20), np.float16)],
                    axis=0,
                )
                for _ in range(4)
            ],
            axis=0,
        ),  # [128,20]
        "b1m": np.tile(b1, 8).reshape(128, 1),
        "b2m": np.tile(b2p, 4).reshape(128, 1),
        "b3m": np.tile(b3p, 2).reshape(128, 1),
        "bo": inputs["bo"].reshape(10, 1).astype(np.float32),
        "a2v": a2[:400].reshape(16, 25).sum(axis=1).reshape(16, 1),
        "a3v": a3[:500].reshape(20, 25).sum(axis=1).reshape(20, 1),
        "ones16": np.ones((16, 1), np.float32),
        "ones20": np.ones((20, 1), np.float32),
        "tc2": np.array(
            [[0.5 * a2[400:].sum() + inputs["c2"].astype(np.float32)[0]]], np.float32
        ),
        "tc3": np.array(
            [[0.5 * a3[500:].sum() + inputs["c3"].astype(np.float32)[0]]], np.float32
        ),
        "kh2t": np.tile(2.0 * kh2p - padco(np.ones(20, np.float32), 32), 4)
        .reshape(1, 128)
        .astype(np.float16),
        "kh2ct": np.tile(kh2cp, 4).reshape(128, 1),
        "kh3t": np.tile(2.0 * kh3p - padco(np.ones(20, np.float32), 64), 2)
        .reshape(1, 128)
        .astype(np.float16),
        "kh3ct": np.tile(kh3cp, 2).reshape(128, 1),
    }
    in_maps = []
    for i in range(NCORES):
        xs = x[i * B : (i + 1) * B]
        m = dict(shared)
        m["g"] = _im2col1(xs)
        # [128 = px8*16+ci, B] mask for H1 tiles (same pattern both tiles)
        m["mm1h"] = np.tile(m1[i * B : (i + 1) * B].T, (8, 1)).astype(np.float16)
        in_maps.append(m)
    return in_maps


# ---------------------------------------------------------------- device build
def _parity_ge1(nc, pool, t_ap, C, outtile, eng=None):
    """outtile = (floor(t) mod 2) as 0/1 via fp32 magic rounding."""
    e = eng if eng is not None else nc.vector
    MAGIC = 12582912.0  # 1.5 * 2^23
    a = pool.tile([C, t_ap.shape[1]], F32, tag="par_a", name="par_a")
    e.tensor_scalar_mul(a[:], t_ap, 0.5)
    e.tensor_scalar_add(a[:], a[:], -0.5)
    e.tensor_scalar_add(a[:], a[:], MAGIC)
    e.tensor_scalar_add(a[:], a[:], -MAGIC)  # a = floor(t/2)
    u = pool.tile([C, t_ap.shape[1]], F32, tag="par_u", name="par_u")
    e.tensor_scalar_mul(u[:], a[:], -2.0)
    e.tensor_tensor(u[:], u[:], t_ap, ALU.add)
    e.tensor_single_scalar(outtile, u[:], 1.0, ALU.is_ge)


def build_kernel():
    nc = bacc.Bacc(None, target_bir_lowering=False, debug=False)

    def din(name, shape, dtype=F32):
        return nc.dram_tensor(name, list(shape), dtype, kind="ExternalInput").ap()

    g_in = din("g", (4, 128, 36, B), F16)
    l1a_in = din("l1a", (128, 128), F16)
    l1b_in = din("l1b", (128, 128), F16)
    l2_in = din("l2", (5, 128, 128), F16)
    l2e0_in = din("l2e0", (5, 128, 128), F16)
    l2e3_in = din("l2e3", (5, 128, 128), F16)
    l3_in = din("l3", (5, 128, 128), F16)
    l3e0_in = din("l3e0", (5, 128, 128), F16)
    l3e3_in = din("l3e3", (5, 128, 128), F16)
    lo_in = din("lo", (64, 160), F16)
    s2b_in = din("s2b", (128, 16), F16)
    s3b_in = din("s3b", (128, 20), F16)
    b1m_in = din("b1m", (128, 1))
    b2m_in = din("b2m", (128, 1))
    b3m_in = din("b3m", (128, 1))
    bo_in = din("bo", (10, 1))
    a2v_in = din("a2v", (16, 1))
    a3v_in = din("a3v", (20, 1))
    ones16_in = din("ones16", (16, 1))
    ones20_in = din("ones20", (20, 1))
    tc2_in = din("tc2", (1, 1))
    tc3_in = din("tc3", (1, 1))
    kh2t_in = din("kh2t", (1, 128), F16)
    kh2ct_in = din("kh2ct", (128, 1))
    kh3t_in = din("kh3t", (1, 128), F16)
    kh3ct_in = din("kh3ct", (128, 1))
    mm1h_in = din("mm1h", (128, B), F16)
    out = nc.dram_tensor("out", [10, B], F32, kind="ExternalOutput").ap()

    with tile.TileContext(nc) as tc:
        with (
            tc.tile_pool(name="const", bufs=1) as cpool,
            tc.tile_pool(name="g", bufs=3) as gpool,
            tc.tile_pool(name="h", bufs=1) as hpool,
            tc.tile_pool(name="rhs", bufs=2) as rpool,
            tc.tile_pool(name="apool", bufs=3) as apool,
            tc.tile_pool(name="ppool", bufs=3) as ppool,
            tc.tile_pool(name="mvpool", bufs=2) as mvpool,
            tc.tile_pool(name="q", bufs=1) as qpool,
            tc.tile_pool(name="cps", bufs=3, space="PSUM") as cps,
            tc.tile_pool(name="acc", bufs=2, space="PSUM") as acc,
        ):
            def load_const(ap, dtype, tag, eng=None):
                t = cpool.tile(list(ap.shape), dtype, tag=tag, name=tag)
                (eng or nc.scalar).dma_start(t[:], ap[:])
                return t

            L1A = load_const(l1a_in, F16, "l1a")
            L1B = load_const(l1b_in, F16, "l1b")
            B1 = load_const(b1m_in, F32, "b1m")

            _gload_n = [0]

            def load_g(c, half):
                """Half-chunk: cols y in [0,20) (half 0) or [16,36) (half 1).
                First 3 loads (one per slot) write the zero rows 108:128;
                later loads skip them — the slot keeps the zeros."""
                t = gpool.tile([128, 20 * B], F16, tag="g", name=f"g{c}_{half}")
                tv = t[:].rearrange("p (y b) -> p y b", y=20)
                yb = 16 * half
                nrow = 128 if _gload_n[0] < 3 else 108
                blocks = ((0, 4), (4, 8), (8, 14), (14, 20)) if _gload_n[0] == 0 else (
                    (0, 8), (8, 14), (14, 20)
                )
                _gload_n[0] += 1
                for y0, y1 in blocks:
                    nc.sync.dma_start(
                        tv[0:nrow, y0:y1, :], g_in[c, 0:nrow, yb + y0 : yb + y1, :]
                    )
                return t

            gtiles = [load_g(0, 0), load_g(0, 1)]

            MM1H = load_const(mm1h_in, F16, "mm1h")
            S2B = load_const(s2b_in, F16, "s2b")
            S3B = load_const(s3b_in, F16, "s3b")
            B2 = load_const(b2m_in, F32, "b2m")
            B3 = load_const(b3m_in, F32, "b3m")
            BO = load_const(bo_in, F32, "bo")
            A2V = load_const(a2v_in, F32, "a2v")
            A3V = load_const(a3v_in, F32, "a3v")
            ON16 = load_const(ones16_in, F32, "ones16")
            ON20 = load_const(ones20_in, F32, "ones20")
            TC2 = load_const(tc2_in, F32, "tc2")
            TC3 = load_const(tc3_in, F32, "tc3")
            KH2T = load_const(kh2t_in, F16, "kh2t")
            KH2CT = load_const(kh2ct_in, F32, "kh2ct")
            KH3T = load_const(kh3t_in, F16, "kh3t")
            KH3CT = load_const(kh3ct_in, F32, "kh3ct")
            L2 = [load_const(l2_in[k], F16, f"l2_{k}", nc.gpsimd) for k in range(5)]
            L2E0 = [load_const(l2e0_in[k], F16, f"l2e0_{k}", nc.gpsimd) for k in range(5)]
            L2E3 = [load_const(l2e3_in[k], F16, f"l2e3_{k}", nc.gpsimd) for k in range(5)]
            L3 = [load_const(l3_in[k], F16, f"l3_{k}", nc.gpsimd) for k in range(5)]
            L3E0 = [load_const(l3e0_in[k], F16, f"l3e0_{k}", nc.gpsimd) for k in range(5)]
            L3E3 = [load_const(l3e3_in[k], F16, f"l3e3_{k}", nc.gpsimd) for k in range(5)]
            LO = load_const(lo_in, F16, "lo", nc.gpsimd)

            H1 = [
                hpool.tile([128, 16 * B], F16, tag=f"h1_{i}", name=f"h1_{i}")
                for i in range(2)
            ]
            H2 = [
                hpool.tile([128, 8 * B], F16, tag=f"h2_{i}", name=f"h2_{i}")
                for i in range(2)
            ]
            H3 = [
                hpool.tile([64, 4 * B], F16, tag=f"h3_{d}", name=f"h3_{d}")
                for d in range(4)
            ]

            r2 = {}

            def stage2(d):
                rhs = rpool.tile([128, 16 * B], F16, tag="rhs2", name=f"rhs2_{d}")
                if d == 0:
                    nc.sync.dma_start(rhs[0:96, :], H1[0][0:96, :])
                    nc.sync.dma_start(rhs[96:128, :], H1[0][0:32, :])
                elif d == 1:
                    nc.sync.dma_start(rhs[0:96, :], H1[0][32:128, :])
                    nc.sync.dma_start(rhs[96:128, :], H1[1][0:32, :])
                elif d == 2:
                    nc.sync.dma_start(rhs[0:32, :], H1[0][96:128, :])
                    nc.sync.dma_start(rhs[32:128, :], H1[1][0:96, :])
                else:
                    nc.sync.dma_start(rhs[0:96, :], H1[1][32:128, :])
                r2[("c2", d)] = rhs

            def stage3(d):
                rhs = rpool.tile([128, 16 * B], F16, tag="rhs2", name=f"rhs3_{d}")
                wxs = range(2, 6) if d == 0 else (range(0, 4) if d == 3 else range(6))
                for r, wx in enumerate(wxs):
                    px3 = 2 * d - 2 + wx
                    src = H2[px3 // 4]
                    p0 = (px3 % 4) * 32
                    nc.sync.dma_start(
                        rhs[r * 20 : (r + 1) * 20, 0 : 8 * B], src[p0 : p0 + 20, :]
                    )
                r2[("c3", d)] = rhs

            def mask_h1_piece(i, j):
                """Multiply mask into H1[i] cols [4j,4j+4) (4 of 16 oy)."""
                h1v = H1[i][:, 4 * j * B : (4 * j + 4) * B].rearrange(
                    "p (y b) -> p y b", y=4
                )
                mb = MM1H[:].broadcast_to((128, B, 4)).rearrange("p b y -> p y b")
                nc.vector.tensor_tensor(h1v, h1v, mb, ALU.mult)

            def cm2_block(i, cm2_ps):
                for oy in range(16):
                    nc.tensor.matmul(
                        cm2_ps[:],
                        S2B[:],
                        H1[i][:, oy * B : (oy + 1) * B],
                        start=(i == 0 and oy == 0),
                        stop=(i == 1 and oy == 15),
                        skip_group_check=True,
                    )

            def mask_h2(i, mm2):
                for j in range(2):
                    h2v = H2[i][:, 4 * j * B : (4 * j + 4) * B].rearrange(
                        "p (y b) -> p y b", y=4
                    )
                    mb = mm2[:].broadcast_to((128, B, 4)).rearrange("p b y -> p y b")
                    nc.vector.tensor_tensor(h2v, h2v, mb, ALU.mult)

            def cm3_block(i, cm3_ps):
                for oy in range(8):
                    nc.tensor.matmul(
                        cm3_ps[:],
                        S3B[:],
                        H2[i][:, oy * B : (oy + 1) * B],
                        start=(i == 0 and oy == 0),
                        stop=(i == 1 and oy == 7),
                        skip_group_check=True,
                    )

            def qchain_pre(cmps_ap, C, lname, mask_ap=None):
                """ACT copies + optional mask + square — emitted right after
                the cm accumulation stops so they run a phase early."""
                cmsb = qpool.tile([C, B], F32, tag="q_cmsb", name=f"cmsb{lname}")
                nc.scalar.activation(cmsb[:], cmps_ap, AF.Identity)
                if mask_ap is not None:
                    nc.vector.tensor_tensor(cmsb[:], cmsb[:], mask_ap, ALU.mult)
                sq = qpool.tile([C, B], F32, tag="q_sq", name=f"sq{lname}")
                nc.scalar.activation(sq[:], cmsb[:], AF.Square)
                return cmsb, sq

            def qchain_a(pre, C, AV, ONESC, TC, lname):
                cmsb, sq = pre
                den_ps = acc.tile([1, B], F32, tag="acc", name=f"den{lname}")
                nc.tensor.matmul(den_ps[:], ONESC[:, 0:1], sq[:], start=True, stop=True)
                num_ps = acc.tile([1, B], F32, tag="acc", name=f"num{lname}")
                nc.tensor.matmul(num_ps[:], AV[:, 0:1], cmsb[:], start=True, stop=True)
                den = qpool.tile([1, B], F32, tag="q_den", name=f"den{lname}")
                nc.scalar.activation(den[:], den_ps[:], AF.Sqrt, scale=25.0)
                nc.vector.tensor_scalar_add(den[:], den[:], EPS)
                rden = qpool.tile([1, B], F32, tag="q_rden", name=f"rden{lname}")
                scr = qpool.tile([1, B], F32, tag="q_qv", name=f"scr{lname}")
                nc.vector.reciprocal_approx_accurate(rden[:], den[:], scr[:])
                nums = qpool.tile([1, B], F32, tag="q_nums", name=f"nums{lname}")
                nc.scalar.activation(nums[:], num_ps[:], AF.Identity)
                qv = qpool.tile([1, B], F32, tag="q_qv", name=f"qv{lname}")
                nc.vector.tensor_tensor(qv[:], nums[:], rden[:], ALU.mult)
                nc.vector.tensor_scalar(qv[:], qv[:], TC[0:1, 0:1], 1.0 / R, ALU.add, ALU.mult)
                # fused parity: floor(t) mod 2 via magic rounding, fp16 0/1 out
                MAGIC = 12582912.0  # 1.5 * 2^23
                pa = qpool.tile([1, B], F32, tag="q_pa", name=f"pa{lname}")
                nc.vector.tensor_scalar(pa[:], qv[:], 0.5, -0.5, ALU.mult, ALU.add)
                nc.vector.tensor_scalar(pa[:], pa[:], MAGIC, -MAGIC, ALU.add, ALU.add)
                pu = qpool.tile([1, B], F32, tag="q_pu", name=f"pu{lname}")
                nc.vector.scalar_tensor_tensor(pu[:], pa[:], -2.0, qv[:], ALU.mult, ALU.add)
                qh = qpool.tile([1, B], F16, tag="q_qh", name=f"qh{lname}")
                nc.vector.tensor_single_scalar(qh[:], pu[:], 1.0, ALU.is_ge)
                return qh, None

            def qchain_b(qh, qc, KHT, KHCT, lname):
                # mask = (2kh-1)*qh + (1-kh): rank-1 fp16 matmul + per-row bias
                map_ps = acc.tile([128, B], F32, tag="acc", name=f"map{lname}")
                nc.tensor.matmul(map_ps[:], KHT[0:1, :], qh[:], start=True, stop=True)
                mm = hpool.tile([128, B], F16, tag=f"mm{lname}", name=f"mm{lname}")
                nc.scalar.activation(mm[:], map_ps[:], AF.Identity, bias=KHCT[:])
                return mm

            pend = [None]  # delayed x-fold: (dst_ap, pp, mv)

            def flush_fold():
                if pend[0] is not None:
                    dst, fpp, fmv = pend[0]
                    nc.vector.tensor_tensor(dst, fpp[0:64, :], fmv[:], ALU.max)
                    pend[0] = None

            cm2_ps = acc.tile([16, B], F32, tag="acc", name="cm2_ps")

            # ---------------- conv1
            for hc in range(8):
                c, half = hc // 2, hc % 2
                gt = gtiles[hc]
                if hc < 6:
                    gtiles.append(load_g((hc + 2) // 2, (hc + 2) % 2))
                if hc == 5:
                    cm2_block(0, cm2_ps)   # H1[0] masked during hc4
                    stage2(0)
                for e in range(4 * half, 4 * half + 4):
                    pp = ppool.tile([128, 2 * B], F16, tag="pp", name="pp")
                    for oy2 in range(2):
                        oy = 4 * e + 2 * oy2
                        yoff = 16 * half  # tile col = y - yoff
                        ps = cps.tile([128, 2 * B], F32, tag="cps", name="cps")
                        gv = gt[:].rearrange("p (y b) -> p y b", y=20)
                        for sub in range(2):
                            nc.tensor.matmul(
                                ps[:, sub * B : (sub + 1) * B],
                                L1A[:],
                                gv[:, oy + sub - yoff, :],
                                start=True,
                                stop=False,
                            )
                            nc.tensor.matmul(
                                ps[:, sub * B : (sub + 1) * B],
                                L1B[:],
                                gv[:, oy + sub + 3 - yoff, :],
                                start=False,
                                stop=True,
                            )
                        a = apool.tile([128, 2 * B], F16, tag="act", name="act")
                        nc.scalar.activation(a[:], ps[:], AF.Relu, bias=B1[:])
                        nc.vector.tensor_tensor(
                            pp[:, oy2 * B : (oy2 + 1) * B],
                            a[:, 0:B],
                            a[:, B : 2 * B],
                            ALU.max,
                        )
                    mv = mvpool.tile([64, 2 * B], F16, tag="mv", name="mv")
                    nc.gpsimd.dma_start(mv[:], pp[64:128, :])
                    flush_fold()
                    # chunk c covers px 4c..4c+3 -> H1[c//2] at offset (c%2)*64
                    pend[0] = (
                        H1[c // 2][
                            (c % 2) * 64 : (c % 2) * 64 + 64,
                            2 * e * B : (2 * e + 2) * B,
                        ],
                        pp,
                        mv,
                    )
                    if hc == 4:
                        mask_h1_piece(0, e)
            flush_fold()
            for j in range(4):
                mask_h1_piece(1, j)
            stage2(1)

            # ---------------- conv2 (trailing ops of each d emitted one d later)
            cm3_ps = None
            mm2 = None
            qh2 = qc2 = None
            for d in range(4):
                rhs = r2[("c2", d)]

                def lhs2(ky, d=d):
                    if d == 0:
                        return L2E0[ky][:]
                    if d == 3:
                        return L2E3[ky][:]
                    return L2[ky][:]

                rv = rhs[:, :].rearrange("p (y b) -> p y b", y=16)
                for oy2 in range(8):
                    ps = cps.tile([128, 2 * B], F32, tag="cps", name="cps")
                    for sub in range(2):
                        oy = 2 * oy2 + sub
                        kys = [k for k in range(5) if 0 <= oy + k - 2 < 16]
                        for j, ky in enumerate(kys):
                            nc.tensor.matmul(
                                ps[:, sub * B : (sub + 1) * B],
                                lhs2(ky),
                                rv[:, oy + ky - 2, :],
                                start=(j == 0),
                                stop=(j == len(kys) - 1),
                            )
                    a = apool.tile([128, 2 * B], F16, tag="act", name="act")
                    nc.scalar.activation(a[:], ps[:], AF.Relu, bias=B2[:])
                    if oy2 % 2 == 0:
                        pp = ppool.tile([128, 2 * B], F16, tag="pp", name="pp")
                    nc.vector.tensor_tensor(
                        pp[:, (oy2 % 2) * B : (oy2 % 2 + 1) * B],
                        a[:, 0:B],
                        a[:, B : 2 * B],
                        ALU.max,
                    )
                    if oy2 % 2 == 1:
                        mv = mvpool.tile([64, 2 * B], F16, tag="mv", name="mv")
                        nc.gpsimd.dma_start(mv[:], pp[64:128, :])
                        flush_fold()
                        j2 = oy2 // 2  # pooled-row pair index
                        pend[0] = (
                            H2[d // 2][
                                (d % 2) * 64 : (d % 2) * 64 + 64,
                                2 * j2 * B : (2 * j2 + 2) * B,
                            ],
                            pp,
                            mv,
                        )
                if d == 0:
                    cm2_block(1, cm2_ps)
                    pre2 = qchain_pre(cm2_ps[:], 16, "2")
                    stage2(2)
                elif d == 1:
                    qh2, qc2 = qchain_a(pre2, 16, A2V, ON16, TC2, "2")
                    stage2(3)
                elif d == 2:
                    mm2 = qchain_b(qh2, qc2, KH2T, KH2CT, "2")
                    mask_h2(0, mm2)
                    cm3_ps = acc.tile([20, B], F32, tag="acc", name="cm3_ps")
                    cm3_block(0, cm3_ps)
                    stage3(0)
            flush_fold()
            mask_h2(1, mm2)
            stage3(1)
            pre3 = None

            # ---------------- conv3
            mm3 = None
            qh3 = qc3 = None
            fc_ps = None

            def mask_fc(dm, half=None):
                oyps = range(4) if half is None else range(2 * half, 2 * half + 2)
                c0, c1 = (0, 4 * B) if half is None else (2 * half * B, (2 * half + 2) * B)
                h3v = H3[dm][:, c0:c1].rearrange("p (y b) -> p y b", y=2 if half is not None else 4)
                nyy = 2 if half is not None else 4
                mb = (
                    mm3[0:64, :].broadcast_to((64, B, nyy)).rearrange("p b y -> p y b")
                )
                nc.vector.tensor_tensor(h3v, h3v, mb, ALU.mult)
                for oyp in oyps:
                    nc.tensor.matmul(
                        fc_ps[:],
                        LO[:, (dm * 4 + oyp) * 10 : (dm * 4 + oyp) * 10 + 10],
                        H3[dm][:, oyp * B : (oyp + 1) * B],
                        start=(dm == 0 and oyp == 0),
                        stop=(dm == 3 and oyp == 3),
                        skip_group_check=True,
                    )
            for d in range(4):
                rhs = r2[("c3", d)]

                def lhs3(ky, d=d):
                    if d == 0:
                        return L3E0[ky][:]
                    if d == 3:
                        return L3E3[ky][:]
                    return L3[ky][:]

                rv = rhs[:, 0 : 8 * B].rearrange("p (y b) -> p y b", y=8)
                for oy2 in range(4):
                    ps = cps.tile([128, 2 * B], F32, tag="cps", name="cps")
                    for sub in range(2):
                        oy = 2 * oy2 + sub
                        kys = [k for k in range(5) if 0 <= oy + k - 2 < 8]
                        for j, ky in enumerate(kys):
                            nc.tensor.matmul(
                                ps[:, sub * B : (sub + 1) * B],
                                lhs3(ky),
                                rv[:, oy + ky - 2, :],
                                start=(j == 0),
                                stop=(j == len(kys) - 1),
                            )
                    a = apool.tile([128, 2 * B], F16, tag="act", name="act")
                    nc.scalar.activation(a[:], ps[:], AF.Relu, bias=B3[:])
                    if oy2 % 2 == 0:
                        pp = ppool.tile([128, 2 * B], F16, tag="pp", name="pp")
                    nc.vector.tensor_tensor(
                        pp[:, (oy2 % 2) * B : (oy2 % 2 + 1) * B],
                        a[:, 0:B],
                        a[:, B : 2 * B],
                        ALU.max,
                    )
                    if oy2 % 2 == 1:
                        mv = mvpool.tile([64, 2 * B], F16, tag="mv", name="mv")
                        nc.gpsimd.dma_start(mv[:], pp[64:128, :])
                        flush_fold()
                        j2 = oy2 // 2
                        pend[0] = (
                            H3[d][:, 2 * j2 * B : (2 * j2 + 2) * B],
                            pp,
                            mv,
                        )
                if d == 0:
                    stage3(2)
                elif d == 1:
                    qh3, qc3 = qchain_a(pre3, 20, A3V, ON20, TC3, "3")
                    stage3(3)
                elif d == 2:
                    mm3 = qchain_b(qh3, qc3, KH3T, KH3CT, "3")
                    fc_ps = acc.tile([10, B], F32, tag="acc", name="fc_ps")
                    mask_fc(0)
            flush_fold()
            mask_fc(3, half=1)

            ob = qpool.tile([10, B], F32, tag="outb", name="outb")
            nc.scalar.activation(ob[:], fc_ps[:], AF.Identity, bias=BO[:])
            nc.sync.dma_start(out[:], ob[:])

    nc.compile()
    return nc


# ---------------------------------------------------------------- entry point
def kernel(**inputs) -> np.ndarray:
    in_maps = _host_prep(inputs)
    if "nc" not in _CACHED:
        _CACHED["nc"] = build_kernel()
    nc = _CACHED["nc"]
    res = run_bass_kernel_spmd(nc, in_maps, core_ids=list(range(NCORES)))
    outs = [res.results[i]["out"].T for i in range(NCORES)]
    return np.ascontiguousarray(np.concatenate(outs, axis=0)).astype(np.float32)
